# revision 14
# baseline (speedup 1.0000x reference)
"""Trainium2 Bass kernel for LayerNorm + multi-head attention + out-projection.

Reference computation (f32):
    h = LayerNorm(x) * ln_w + ln_b
    q, k, v = split(h @ w_qkv)          # 16 heads, head_dim 64
    out = softmax(q k^T / 8) v          # per head, full 2048-seq attention
    return concat_heads(out) @ w_out

Sharding over 8 NeuronCores: core c -> (batch b = c // 2, head-group g = c % 2).
Each core handles one batch and 8 of the 16 heads (tensor parallel on heads:
w_qkv column-split, w_out row-split).  Each core emits a partial [2048, 1024]
output; the host sums the two partials of each batch (the all-reduce is a
2-element sum done on host after gather).

Device-side dataflow per core (all matmuls out = lhsT.T @ rhs):
  - LayerNorm in token-major layout (bn_stats/bn_aggr, per-partition scalars)
  - PE-transpose h -> hT [d on partitions, tokens free] (bf16)
  - qT = Wq.T @ hT, kT = Wk.T @ hT   (f32; [head cols on partitions, tokens])
  - V  = hT.T @ Wv                   (bf16; [tokens, cols] natural, + a ones
                                      column per head for softmax denominators)
  - per (head, q-block): S^T tile = kT_h.T @ qT_h  -> exp on ScalarE (scale
    1/8 fused; S ~ N(0,1) so no max subtraction needed) -> O' = [V_h|1].T @ P^T
    accumulated over k tiles: rows 0..63 = unnormalized attn out (transposed),
    row 64 = softmax denominator.  Normalize with DVE using a DMA-broadcast
    reciprocal, into oT (bf16).
  - out = oT.T @ Wout  (natural layout, DMA to DRAM)
"""

import os
import sys
from contextlib import ExitStack

import numpy as np

import concourse.bass as bass
import concourse.tile as tile
from concourse import bacc, mybir
from concourse.masks import make_identity

import ml_dtypes

P = 128
EPS = 1e-5


def _bcast_partition(ap, n, skip_partition=True):
    """AP that reads a [1, F] access pattern broadcast to [n, F] partitions.

    skip_partition: drop the existing (size-1) partition dim of an on-chip AP;
    False for DRAM APs, whose dims are all kept as free dims.
    """
    dims = list(ap.ap[1:]) if skip_partition else list(ap.ap)
    if skip_partition:
        # SBUF source: partition step 0 is illegal, so read [1, n, F] from the
        # single source partition (step-0 repeat in a free dim) and let the
        # DMA scatter across destination partitions.
        part = list(ap.ap[0])
        return bass.AP(tensor=ap.tensor, offset=ap.offset,
                       ap=[[part[0], 1], [0, n]] + dims)
    return bass.AP(tensor=ap.tensor, offset=ap.offset, ap=[[0, n]] + dims)


def emit_body(ctx, tc, io, ntok, d, nh, hd):
    nc = tc.nc
    f32 = mybir.dt.float32
    bf16 = mybir.dt.bfloat16
    f32r = mybir.dt.float32r
    Act = mybir.ActivationFunctionType
    Alu = mybir.AluOpType

    cc = nh * hd            # head cols per core (512)
    n_dt = d // P           # d-model tiles (8)
    n_tt = ntok // P        # token tiles (16)
    FQ = min(512, ntok)     # q block / matmul moving size
    n_qb = ntok // FQ       # q blocks (4)
    FN = min(512, d)        # out-proj free block
    n_nb = d // FN          # out-proj col blocks (2)
    n_ct = cc // P          # head-pair tiles (4)
    bn_ch = min(512, d)     # bn_stats chunk size
    n_ch = d // bn_ch       # bn_stats chunks (2)
    vw = hd + 1             # V cols per head incl. ones column (65)
    scale = float(hd) ** -0.5

    x_d, wq_d, wk_d, wv_d, wo_d, lnw_d, lnb_d, out_d = io

    # ---------------- constants & weights ----------------
    const = ctx.enter_context(tc.tile_pool(name="const", bufs=1))
    ident = const.tile([P, P], f32)
    make_identity(nc, ident[:])
    lnw_sb = const.tile([P, d], f32)
    nc.sync.dma_start(out=lnw_sb[:],
                      in_=_bcast_partition(lnw_d, P, skip_partition=False))
    lnb_sb = const.tile([P, d], f32)
    nc.sync.dma_start(out=lnb_sb[:],
                      in_=_bcast_partition(lnb_d, P, skip_partition=False))
    eps_sb = const.tile([P, 1], f32)
    nc.vector.memset(eps_sb[:], EPS)

    wpool = ctx.enter_context(tc.tile_pool(name="weights", bufs=1))
    wq_sb = [wpool.tile([P, cc], bf16, tag=f"wq{k}", name=f"wq{k}") for k in range(n_dt)]
    wk_sb = [wpool.tile([P, cc], bf16, tag=f"wk{k}", name=f"wk{k}") for k in range(n_dt)]
    wv_sb = [wpool.tile([P, cc], bf16, tag=f"wv{k}", name=f"wv{k}") for k in range(n_dt)]
    wo_sb = [wpool.tile([P, d], bf16, tag=f"wo{j}", name=f"wo{j}") for j in range(n_ct)]
    for k in range(n_dt):
        nc.sync.dma_start(out=wq_sb[k][:], in_=wq_d[k * P:(k + 1) * P, :])
        nc.sync.dma_start(out=wk_sb[k][:], in_=wk_d[k * P:(k + 1) * P, :])
        nc.sync.dma_start(out=wv_sb[k][:], in_=wv_d[k * P:(k + 1) * P, :])
    for j in range(n_ct):
        nc.sync.dma_start(out=wo_sb[j][:], in_=wo_d[j * P:(j + 1) * P, :])

    # ---------------- persistent activations ----------------
    big = ctx.enter_context(tc.tile_pool(name="big", bufs=1))
    hT = [big.tile([P, ntok], bf16, tag=f"hT{k}", name=f"hT{k}") for k in range(n_dt)]
    qT = [big.tile([P, ntok], f32r, tag=f"qT{j}", name=f"qT{j}") for j in range(n_ct)]
    kT = [big.tile([P, ntok], f32r, tag=f"kT{j}", name=f"kT{j}") for j in range(n_ct)]
    V = [big.tile([P, nh * vw], bf16, tag=f"V{t}", name=f"V{t}") for t in range(n_tt)]
    oT = [big.tile([P, ntok], bf16, tag=f"oT{j}", name=f"oT{j}") for j in range(n_ct)]

    # ---------------- phase 1: LayerNorm + transpose ----------------
    with tc.tile_pool(name="xin", bufs=3) as xin_p, \
         tc.tile_pool(name="hnat", bufs=3) as h_p, \
         tc.tile_pool(name="stats", bufs=6) as st_p, \
         tc.tile_pool(name="ptr", bufs=2, space="PSUM") as ptr_p:
        for t in range(n_tt):
            xt = xin_p.tile([P, d], f32, tag="xt")
            nc.sync.dma_start(out=xt[:], in_=x_d[t * P:(t + 1) * P, :])
            st = st_p.tile([P, n_ch, 6], f32, tag="st")
            for c in range(n_ch):
                nc.vector.bn_stats(st[:, c, :], xt[:, c * bn_ch:(c + 1) * bn_ch])
            mv = st_p.tile([P, 2], f32, tag="mv")
            nc.vector.bn_aggr(mv[:], st[:])
            rstd = st_p.tile([P, 1], f32, tag="rstd")
            nc.scalar.activation(rstd[:], mv[:, 1:2], Act.Sqrt,
                                 bias=eps_sb[:], scale=1.0)
            nc.vector.reciprocal(rstd[:], rstd[:])
            ht = h_p.tile([P, d], f32, tag="ht")
            nc.vector.tensor_scalar(out=ht[:], in0=xt[:],
                                    scalar1=mv[:, 0:1], scalar2=rstd[:],
                                    op0=Alu.subtract, op1=Alu.mult)
            nc.vector.tensor_mul(ht[:], ht[:], lnw_sb[:])
            nc.vector.tensor_add(ht[:], ht[:], lnb_sb[:])
            for g in range(0, n_dt, 4):
                ng = min(4, n_dt - g)
                ps = ptr_p.tile([P, 512], f32, tag="ptr")
                for jj in range(ng):
                    nc.tensor.transpose(ps[:, jj * P:(jj + 1) * P],
                                        ht[:, (g + jj) * P:(g + jj + 1) * P],
                                        ident[:])
                for jj in range(ng):
                    nc.vector.tensor_copy(out=hT[g + jj][:, t * P:(t + 1) * P],
                                          in_=ps[:, jj * P:(jj + 1) * P])

    # ---------------- phase 2: QKV projections ----------------
    with tc.tile_pool(name="psq", bufs=3, space="PSUM") as psq_p:
        for t in range(n_tt):
            vv = V[t][:].rearrange("p (h c) -> p h c", c=vw)
            nc.vector.memset(vv[:, :, hd:hd + 1], 1.0)
            ps = psq_p.tile([P, cc], f32, tag="psq")
            for k in range(n_dt):
                nc.tensor.matmul(ps[:], lhsT=hT[k][:, t * P:(t + 1) * P],
                                 rhs=wv_sb[k][:],
                                 start=(k == 0), stop=(k == n_dt - 1))
            nc.vector.tensor_copy(out=vv[:, :, 0:hd],
                                  in_=ps[:].rearrange("p (h c) -> p h c", c=hd))
        for dst, w_sb in ((qT, wq_sb), (kT, wk_sb)):
            for j in range(n_ct):
                for tb in range(n_qb):
                    ps = psq_p.tile([P, FQ], f32, tag="psq")
                    for k in range(n_dt):
                        nc.tensor.matmul(ps[:], lhsT=w_sb[k][:, j * P:(j + 1) * P],
                                         rhs=hT[k][:, tb * FQ:(tb + 1) * FQ],
                                         start=(k == 0), stop=(k == n_dt - 1))
                    nc.vector.tensor_copy(out=dst[j][:, tb * FQ:(tb + 1) * FQ],
                                          in_=ps[:])

    # ---------------- phase 3+4: attention + out-projection ----------------
    with tc.tile_pool(name="pss", bufs=4, space="PSUM") as pss_p, \
         tc.tile_pool(name="pso", bufs=2, space="PSUM") as pso_p, \
         tc.tile_pool(name="psout", bufs=2, space="PSUM") as psout_p, \
         tc.tile_pool(name="expp", bufs=4) as exp_p, \
         tc.tile_pool(name="rsp", bufs=4) as rs_p, \
         tc.tile_pool(name="outp", bufs=3) as out_p:
        for qb in range(n_qb):
            for h in range(nh):
                j, off = h // 2, (h % 2) * hd
                po = pso_p.tile([vw, FQ], f32, tag="po")
                for kt in range(n_tt):
                    ps = pss_p.tile([P, FQ], f32, tag="pss")
                    nc.tensor.matmul(
                        ps[:],
                        lhsT=kT[j][off:off + hd, kt * P:(kt + 1) * P],
                        rhs=qT[j][off:off + hd, qb * FQ:(qb + 1) * FQ],
                        start=True, stop=True)
                    ex = exp_p.tile([P, FQ], bf16, tag="ex")
                    nc.scalar.activation(ex[:], ps[:], Act.Exp, scale=scale)
                    nc.tensor.matmul(po[:], lhsT=V[kt][:, h * vw:(h + 1) * vw],
                                     rhs=ex[:],
                                     start=(kt == 0), stop=(kt == n_tt - 1))
                rs = rs_p.tile([1, FQ], f32, tag="rs")
                nc.vector.reciprocal(rs[:], po[hd:hd + 1, :])
                rsb = rs_p.tile([hd, FQ], f32, tag="rsb")
                nc.sync.dma_start(out=rsb[:], in_=_bcast_partition(rs[:], hd))
                nc.vector.tensor_mul(oT[j][off:off + hd, qb * FQ:(qb + 1) * FQ],
                                     po[0:hd, :], rsb[:])
            for tt in range(qb * FQ // P, (qb + 1) * FQ // P):
                for nb in range(n_nb):
                    ps = psout_p.tile([P, FN], f32, tag="pso2")
                    for j2 in range(n_ct):
                        nc.tensor.matmul(ps[:], lhsT=oT[j2][:, tt * P:(tt + 1) * P],
                                         rhs=wo_sb[j2][:, nb * FN:(nb + 1) * FN],
                                         start=(j2 == 0), stop=(j2 == n_ct - 1))
                    ot = out_p.tile([P, FN], f32, tag="ot")
                    nc.vector.tensor_copy(ot[:], ps[:])
                    nc.sync.dma_start(
                        out=out_d[tt * P:(tt + 1) * P, nb * FN:(nb + 1) * FN],
                        in_=ot[:])


def build_nc(ntok=2048, d=1024, nh=8, hd=64, n_cores=8):
    nc = bacc.Bacc("TRN2", target_bir_lowering=False, debug=False,
                   num_devices=n_cores)
    f32 = mybir.dt.float32
    bf16 = mybir.dt.bfloat16
    cc = nh * hd
    x_d = nc.dram_tensor("x", [ntok, d], f32, kind="ExternalInput").ap()
    wq_d = nc.dram_tensor("wq", [d, cc], bf16, kind="ExternalInput").ap()
    wk_d = nc.dram_tensor("wk", [d, cc], bf16, kind="ExternalInput").ap()
    wv_d = nc.dram_tensor("wv", [d, cc], bf16, kind="ExternalInput").ap()
    wo_d = nc.dram_tensor("wo", [cc, d], bf16, kind="ExternalInput").ap()
    lnw_d = nc.dram_tensor("lnw", [d], f32, kind="ExternalInput").ap()
    lnb_d = nc.dram_tensor("lnb", [d], f32, kind="ExternalInput").ap()
    out_d = nc.dram_tensor("out", [ntok, d], f32, kind="ExternalOutput").ap()
    io = (x_d, wq_d, wk_d, wv_d, wo_d, lnw_d, lnb_d, out_d)
    with tile.TileContext(nc) as tc:
        with ExitStack() as ctx:
            emit_body(ctx, tc, io, ntok, d, nh, hd)
    nc.compile()
    return nc


_CACHE = {}


def _make_runner(nc, n_cores):
    """Build a reusable sharded PJRT callable for the compiled Bass module.

    Mirrors concourse.bass2jax.run_bass_via_pjrt's multi-core path, but holds
    the jitted function so repeated kernel() calls (and timing loops) do not
    re-trace/re-compile the XLA wrapper.
    """
    import jax
    from jax.sharding import Mesh, PartitionSpec
    from jax.experimental.shard_map import shard_map
    from concourse.bass2jax import (_bass_exec_p, install_neuronx_cc_hook,
                                    partition_id_tensor)

    install_neuronx_cc_hook()
    partition_name = (nc.partition_id_tensor.name
                      if nc.partition_id_tensor else None)

    in_names, out_names, out_avals = [], [], []
    for alloc in nc.m.functions[0].allocations:
        if not isinstance(alloc, mybir.MemoryLocationSet):
            continue
        name = alloc.memorylocations[0].name
        if alloc.kind == "ExternalInput":
            if name != partition_name:
                in_names.append(name)
        elif alloc.kind == "ExternalOutput":
            out_names.append(name)
            out_avals.append(jax.core.ShapedArray(
                tuple(alloc.tensor_shape), mybir.dt.np(alloc.dtype)))
    all_names = in_names + out_names
    if partition_name is not None:
        all_names = all_names + [partition_name]

    def _body(*args):
        operands = list(args)
        if partition_name is not None:
            operands.append(partition_id_tensor())
        outs = _bass_exec_p.bind(
            *operands,
            out_avals=tuple(out_avals),
            in_names=tuple(all_names),
            out_names=tuple(out_names),
            lowering_input_output_aliases=(),
            sim_require_finite=True,
            sim_require_nnan=True,
            nc=nc,
        )
        return tuple(outs)

    devices = jax.devices()[:n_cores]
    assert len(devices) == n_cores
    mesh = Mesh(np.asarray(devices), ("core",))
    nio = len(in_names) + len(out_names)
    sharded = jax.jit(
        shard_map(_body, mesh=mesh,
                  in_specs=(PartitionSpec("core"),) * nio,
                  out_specs=(PartitionSpec("core"),) * len(out_names),
                  check_rep=False),
        keep_unused=True)
    return sharded, in_names, out_names, out_avals


def _concat_inputs(in_maps, in_names, out_avals, n_cores):
    concat_in = [np.concatenate([np.asarray(in_maps[c][name])
                                 for c in range(n_cores)], axis=0)
                 for name in in_names]
    concat_zeros = [np.zeros((n_cores * a.shape[0], *a.shape[1:]), a.dtype)
                    for a in out_avals]
    return concat_in + concat_zeros


def _run_spmd(in_maps, n_cores):
    sharded, in_names, out_names, out_avals = _CACHE["runner"]
    args = _concat_inputs(in_maps, in_names, out_avals, n_cores)
    _CACHE["last_args"] = args
    out_arrs = sharded(*args)
    return [
        {name: np.asarray(out_arrs[i]).reshape(n_cores, *out_avals[i].shape)[c]
         for i, name in enumerate(out_names)}
        for c in range(n_cores)
    ]


def kernel(x, ln_w, ln_b, w_qkv, w_out):
    x = np.asarray(x, dtype=np.float32)
    ln_w = np.asarray(ln_w, dtype=np.float32)
    ln_b = np.asarray(ln_b, dtype=np.float32)
    w_qkv = np.asarray(w_qkv, dtype=np.float32)
    w_out = np.asarray(w_out, dtype=np.float32)

    B, ntok, d = x.shape               # 4, 2048, 1024
    inner = w_out.shape[0]             # 1024
    hd = 64
    H = inner // hd                    # 16
    n_cores = 8
    gpb = n_cores // B                 # head-groups per batch (2)
    nh = H // gpb                      # heads per core (8)
    cc = nh * hd                       # 512

    if "nc" not in _CACHE:
        _CACHE["nc"] = build_nc(ntok=ntok, d=d, nh=nh, hd=hd, n_cores=n_cores)
    nc = _CACHE["nc"]

    bf = ml_dtypes.bfloat16
    wq_f = w_qkv[:, 0 * inner:1 * inner]
    wk_f = w_qkv[:, 1 * inner:2 * inner]
    wv_f = w_qkv[:, 2 * inner:3 * inner]

    in_maps = []
    for c in range(n_cores):
        b, g = divmod(c, gpb)
        cols = slice(g * cc, (g + 1) * cc)
        in_maps.append({
            "x": np.ascontiguousarray(x[b]),
            "wq": np.ascontiguousarray(wq_f[:, cols]).astype(bf),
            "wk": np.ascontiguousarray(wk_f[:, cols]).astype(bf),
            "wv": np.ascontiguousarray(wv_f[:, cols]).astype(bf),
            "wo": np.ascontiguousarray(w_out[cols, :]).astype(bf),
            "lnw": ln_w,
            "lnb": ln_b,
        })

    if "runner" not in _CACHE:
        _CACHE["runner"] = _make_runner(nc, n_cores)
    results = _run_spmd(in_maps, n_cores)
    parts = [results[c]["out"] for c in range(n_cores)]
    out = np.stack([sum(parts[b * gpb + g] for g in range(gpb))
                    for b in range(B)])
    return out.astype(np.float32)


# revision 23
# speedup vs baseline: 104.0652x; 104.0652x over previous
"""Trainium2 Bass kernel for LayerNorm + multi-head attention + out-projection.

Reference computation (f32):
    h = LayerNorm(x) * ln_w + ln_b
    q, k, v = split(h @ w_qkv)          # 16 heads, head_dim 64
    out = softmax(q k^T / 8) v          # per head, full 2048-seq attention
    return concat_heads(out) @ w_out

Sharding over 8 NeuronCores: core c -> (batch b = c // 2, head-group g = c % 2).
Each core handles one batch and 8 of the 16 heads (tensor parallel on heads:
w_qkv column-split, w_out row-split).  Each core emits a partial [2048, 1024]
output; the host sums the two partials of each batch (the all-reduce is a
2-element sum done on host after gather).

Device-side dataflow per core (all matmuls out = lhsT.T @ rhs):
  - LayerNorm in token-major layout (bn_stats/bn_aggr, per-partition scalars),
    fused per 128-token tile with PE-transpose h -> hT [d-part, tokens] (bf16)
    and the V projection; kT/qT projections emitted per 512-token block so
    they overlap the next tiles' LayerNorm (float32r so the S matmul runs the
    PE's full-rate reduced-precision fp32 path).
  - V = hT.T @ Wv in natural [tokens, cols] layout with an extra ones column
    per head (accumulates the softmax denominator for free during AV).
  - per (q-block, head-pair): S^T = kT_h.T @ qT_h into a 2-bank PSUM tile
    (two k-tiles wide) -> one 1024-wide exp on ScalarE (1/8 scale fused; S ~
    N(0,1), so no max subtraction is needed for fp32/bf16 range) ->
    O' = [V_h|1].T @ P^T accumulated over k tiles: rows 0..63 = unnormalized
    attention out (transposed), row 64 = denominator.  Head pairs sit at PSUM
    partitions 0/64 so their K=64 S-matmuls can overlap on PE row-groups.
    One PSUM->SBUF copy frees the accumulator, then normalize with DVE using
    a DMA-broadcast reciprocal into oT (bf16).
  - out = oT.T @ Wout (natural layout, streamed to DRAM per q-block).

Engine budget per core (cost model): PE ~352 us, ACT ~268 us (exp-bound
attention), DVE ~228 us; e2e estimate ~475 us.
"""

from contextlib import ExitStack

import numpy as np

import concourse.bass as bass
import concourse.tile as tile
from concourse import bacc, mybir
from concourse.masks import make_identity

import ml_dtypes

P = 128
EPS = 1e-5


def _bcast_partition(ap, n, skip_partition=True):
    """AP that reads a [1, F] access pattern broadcast to [n, F] partitions.

    skip_partition: drop the existing (size-1) partition dim of an on-chip AP;
    False for DRAM APs, whose dims are all kept as free dims.
    """
    dims = list(ap.ap[1:]) if skip_partition else list(ap.ap)
    if skip_partition:
        # SBUF source: partition step 0 is illegal, so read [1, n, F] from the
        # single source partition (step-0 repeat in a free dim) and let the
        # DMA scatter across destination partitions.
        part = list(ap.ap[0])
        return bass.AP(tensor=ap.tensor, offset=ap.offset,
                       ap=[[part[0], 1], [0, n]] + dims)
    return bass.AP(tensor=ap.tensor, offset=ap.offset, ap=[[0, n]] + dims)


def emit_body(ctx, tc, io, ntok, d, nh, hd, repeat=1):
    nc = tc.nc
    f32 = mybir.dt.float32
    bf16 = mybir.dt.bfloat16
    f32r = mybir.dt.float32r
    Act = mybir.ActivationFunctionType
    Alu = mybir.AluOpType

    cc = nh * hd            # head cols per core (512)
    n_dt = d // P           # d-model tiles (8)
    n_tt = ntok // P        # token tiles (16)
    FQ = min(512, ntok)     # q block / matmul moving size
    n_qb = ntok // FQ       # q blocks (4)
    FN = min(512, d)        # out-proj free block
    n_nb = d // FN          # out-proj col blocks (2)
    n_ct = cc // P          # head-pair tiles (4)
    bn_ch = min(512, d)     # bn_stats chunk size
    n_ch = d // bn_ch       # bn_stats chunks (2)
    vw = hd + 1             # V cols per head incl. ones column (65)
    scale = float(hd) ** -0.5

    x_d, wq_d, wk_d, wv_d, wo_d, lnw_d, lnb_d, out_d = io

    # ---------------- constants & weights ----------------
    const = ctx.enter_context(tc.tile_pool(name="const", bufs=1))
    ident = const.tile([P, P], f32)
    make_identity(nc, ident[:])
    lnw_sb = const.tile([P, d], f32)
    nc.gpsimd.dma_start(out=lnw_sb[:],
                        in_=_bcast_partition(lnw_d, P, skip_partition=False))
    lnb_sb = const.tile([P, d], f32)
    nc.gpsimd.dma_start(out=lnb_sb[:],
                        in_=_bcast_partition(lnb_d, P, skip_partition=False))
    eps_sb = const.tile([P, 1], f32)
    nc.vector.memset(eps_sb[:], EPS)

    wpool = ctx.enter_context(tc.tile_pool(name="weights", bufs=1))
    wq_sb = [wpool.tile([P, cc], bf16, tag=f"wq{k}", name=f"wq{k}") for k in range(n_dt)]
    wk_sb = [wpool.tile([P, cc], bf16, tag=f"wk{k}", name=f"wk{k}") for k in range(n_dt)]
    wv_sb = [wpool.tile([P, cc], bf16, tag=f"wv{k}", name=f"wv{k}") for k in range(n_dt)]
    wo_sb = [wpool.tile([P, d], bf16, tag=f"wo{j}", name=f"wo{j}") for j in range(n_ct)]
    for k in range(n_dt):
        nc.gpsimd.dma_start(out=wq_sb[k][:], in_=wq_d[k * P:(k + 1) * P, :])
        nc.gpsimd.dma_start(out=wk_sb[k][:], in_=wk_d[k * P:(k + 1) * P, :])
        nc.gpsimd.dma_start(out=wv_sb[k][:], in_=wv_d[k * P:(k + 1) * P, :])
    for j in range(n_ct):
        nc.gpsimd.dma_start(out=wo_sb[j][:], in_=wo_d[j * P:(j + 1) * P, :])

    # ---------------- persistent activations ----------------
    big = ctx.enter_context(tc.tile_pool(name="big", bufs=1))
    hT = big.tile([P, n_dt, ntok], bf16, tag="hT", name="hT")
    qT = [big.tile([P, ntok], f32r, tag=f"qT{j}", name=f"qT{j}") for j in range(n_ct)]
    kT = [big.tile([P, ntok], f32r, tag=f"kT{j}", name=f"kT{j}") for j in range(n_ct)]
    V = [big.tile([P, nh * vw], bf16, tag=f"V{t}", name=f"V{t}") for t in range(n_tt)]
    oT = [big.tile([P, ntok], bf16, tag=f"oT{j}", name=f"oT{j}") for j in range(n_ct)]

    for _rep in range(repeat):
        _emit_phases(tc, locals())


def _emit_phases(tc, env):
    nc = tc.nc
    f32 = env["f32"]; bf16 = env["bf16"]; f32r = env["f32r"]
    Act = env["Act"]; Alu = env["Alu"]
    ntok = env["ntok"]; d = env["d"]; nh = env["nh"]; hd = env["hd"]
    cc = env["cc"]; n_dt = env["n_dt"]; n_tt = env["n_tt"]
    FQ = env["FQ"]; n_qb = env["n_qb"]; FN = env["FN"]; n_nb = env["n_nb"]
    n_ct = env["n_ct"]; bn_ch = env["bn_ch"]; n_ch = env["n_ch"]
    vw = env["vw"]; scale = env["scale"]
    x_d = env["x_d"]; out_d = env["out_d"]
    eps_sb = env["eps_sb"]; lnw_sb = env["lnw_sb"]; lnb_sb = env["lnb_sb"]
    ident = env["ident"]
    wq_sb = env["wq_sb"]; wk_sb = env["wk_sb"]; wv_sb = env["wv_sb"]
    wo_sb = env["wo_sb"]
    hT = env["hT"]; qT = env["qT"]; kT = env["kT"]; V = env["V"]; oT = env["oT"]

    # -------- phase 1: LayerNorm + transpose + V projection (per t) --------
    with tc.tile_pool(name="xin", bufs=3) as xin_p, \
         tc.tile_pool(name="hnat", bufs=3) as h_p, \
         tc.tile_pool(name="stats", bufs=6) as st_p, \
         tc.tile_pool(name="ptr", bufs=2, space="PSUM") as ptr_p, \
         tc.tile_pool(name="psq", bufs=3, space="PSUM") as psq_p:
        for t in range(n_tt):
            xt = xin_p.tile([P, d], f32, tag="xt")
            nc.sync.dma_start(out=xt[:], in_=x_d[t * P:(t + 1) * P, :])
            st = st_p.tile([P, n_ch, 6], f32, tag="st")
            for c in range(n_ch):
                nc.vector.bn_stats(st[:, c, :], xt[:, c * bn_ch:(c + 1) * bn_ch])
            mv = st_p.tile([P, 2], f32, tag="mv")
            nc.vector.bn_aggr(mv[:], st[:])
            rstd = st_p.tile([P, 1], f32, tag="rstd")
            nc.scalar.activation(rstd[:], mv[:, 1:2], Act.Sqrt,
                                 bias=eps_sb[:], scale=1.0)
            nc.vector.reciprocal(rstd[:], rstd[:])
            ht = h_p.tile([P, d], f32, tag="ht")
            nc.vector.tensor_scalar(out=ht[:], in0=xt[:],
                                    scalar1=mv[:, 0:1], scalar2=rstd[:],
                                    op0=Alu.subtract, op1=Alu.mult)
            nc.vector.tensor_mul(ht[:], ht[:], lnw_sb[:])
            nc.vector.tensor_add(ht[:], ht[:], lnb_sb[:])
            for g in range(0, n_dt, 4):
                ng = min(4, n_dt - g)
                ps = ptr_p.tile([P, 512], f32, tag="ptr")
                for jj in range(ng):
                    nc.tensor.transpose(ps[:, jj * P:(jj + 1) * P],
                                        ht[:, (g + jj) * P:(g + jj + 1) * P],
                                        ident[:])
                nc.vector.tensor_copy(
                    out=hT[:, g:g + ng, t * P:(t + 1) * P],
                    in_=ps[:].rearrange("p (g q) -> p g q", q=P)[:, 0:ng, :])
            # V projection for this token tile (hT for all k just landed)
            vv = V[t][:].rearrange("p (h c) -> p h c", c=vw)
            nc.vector.memset(vv[:, :, hd:hd + 1], 1.0)
            psv = psq_p.tile([P, cc], f32, tag="psq")
            for k in range(n_dt):
                nc.tensor.matmul(psv[:], lhsT=hT[:, k, t * P:(t + 1) * P],
                                 rhs=wv_sb[k][:],
                                 start=(k == 0), stop=(k == n_dt - 1))
            nc.vector.tensor_copy(out=vv[:, :, 0:hd],
                                  in_=psv[:].rearrange("p (h c) -> p h c", c=hd))

            # kT / qT for each completed 512-token block (overlaps next LN)
            if (t + 1) * P % FQ == 0:
                tb = ((t + 1) * P - FQ) // FQ
                for dst, w_sb in ((kT, wk_sb), (qT, wq_sb)):
                    for j in range(n_ct):
                        ps = psq_p.tile([P, FQ], f32, tag="psq")
                        for k in range(n_dt):
                            nc.tensor.matmul(
                                ps[:], lhsT=w_sb[k][:, j * P:(j + 1) * P],
                                rhs=hT[:, k, tb * FQ:(tb + 1) * FQ],
                                start=(k == 0), stop=(k == n_dt - 1))
                        nc.vector.tensor_copy(
                            out=dst[j][:, tb * FQ:(tb + 1) * FQ], in_=ps[:])

    # -------- phase 3+4: attention (head pairs, 1024-wide exp) + out-proj ----
    kt_pair = 2 if n_tt % 2 == 0 else 1
    with tc.tile_pool(name="pss", bufs=2, space="PSUM") as pss_p, \
         tc.tile_pool(name="pso", bufs=2, space="PSUM") as pso_p, \
         tc.tile_pool(name="psout", bufs=2, space="PSUM") as psout_p, \
         tc.tile_pool(name="expp", bufs=4) as exp_p, \
         tc.tile_pool(name="rsp", bufs=3) as rs_p, \
         tc.tile_pool(name="outp", bufs=3) as out_p:
        for qb in range(n_qb):
            for j in range(n_ct):
                pos = [pso_p.tile([vw, FQ], f32, tag="po", name=f"po{hh}")
                       for hh in range(2)]
                for kt2 in range(n_tt // kt_pair):
                    exs = []
                    for hh in range(2):
                        off = hh * hd
                        ps = pss_p.tile([P, kt_pair * FQ], f32, tag="pss")
                        for u in range(kt_pair):
                            kt = kt_pair * kt2 + u
                            nc.tensor.matmul(
                                ps[:, u * FQ:(u + 1) * FQ],
                                lhsT=kT[j][off:off + hd, kt * P:(kt + 1) * P],
                                rhs=qT[j][off:off + hd, qb * FQ:(qb + 1) * FQ],
                                start=True, stop=True)
                        ex = exp_p.tile([P, kt_pair * FQ], bf16, tag="ex")
                        nc.scalar.activation(ex[:], ps[:], Act.Exp, scale=scale)
                        exs.append(ex)
                    for hh in range(2):
                        h = 2 * j + hh
                        for u in range(kt_pair):
                            kt = kt_pair * kt2 + u
                            nc.tensor.matmul(
                                pos[hh][:],
                                lhsT=V[kt][:, h * vw:(h + 1) * vw],
                                rhs=exs[hh][:, u * FQ:(u + 1) * FQ],
                                start=(kt == 0), stop=(kt == n_tt - 1))
                for hh in range(2):
                    h = 2 * j + hh
                    off = hh * hd
                    posb = rs_p.tile([vw, FQ], f32, tag="posb")
                    nc.vector.tensor_copy(posb[:], pos[hh][:])
                    rs = rs_p.tile([1, FQ], f32, tag="rs")
                    nc.vector.reciprocal(rs[:], posb[hd:hd + 1, :])
                    rsb = rs_p.tile([hd, FQ], f32, tag="rsb")
                    nc.sync.dma_start(out=rsb[:], in_=_bcast_partition(rs[:], hd))
                    nc.vector.tensor_mul(
                        oT[j][off:off + hd, qb * FQ:(qb + 1) * FQ],
                        posb[0:hd, :], rsb[:])
            for tt in range(qb * FQ // P, (qb + 1) * FQ // P):
                for nb in range(n_nb):
                    ps = psout_p.tile([P, FN], f32, tag="pso2")
                    for j2 in range(n_ct):
                        nc.tensor.matmul(ps[:], lhsT=oT[j2][:, tt * P:(tt + 1) * P],
                                         rhs=wo_sb[j2][:, nb * FN:(nb + 1) * FN],
                                         start=(j2 == 0), stop=(j2 == n_ct - 1))
                    ot = out_p.tile([P, FN], f32, tag="ot")
                    nc.vector.tensor_copy(ot[:], ps[:])
                    nc.sync.dma_start(
                        out=out_d[tt * P:(tt + 1) * P, nb * FN:(nb + 1) * FN],
                        in_=ot[:])


def build_nc(ntok=2048, d=1024, nh=8, hd=64, n_cores=8, repeat=1):
    nc = bacc.Bacc("TRN2", target_bir_lowering=False, debug=False,
                   num_devices=n_cores)
    f32 = mybir.dt.float32
    bf16 = mybir.dt.bfloat16
    cc = nh * hd
    x_d = nc.dram_tensor("x", [ntok, d], f32, kind="ExternalInput").ap()
    wq_d = nc.dram_tensor("wq", [d, cc], bf16, kind="ExternalInput").ap()
    wk_d = nc.dram_tensor("wk", [d, cc], bf16, kind="ExternalInput").ap()
    wv_d = nc.dram_tensor("wv", [d, cc], bf16, kind="ExternalInput").ap()
    wo_d = nc.dram_tensor("wo", [cc, d], bf16, kind="ExternalInput").ap()
    lnw_d = nc.dram_tensor("lnw", [d], f32, kind="ExternalInput").ap()
    lnb_d = nc.dram_tensor("lnb", [d], f32, kind="ExternalInput").ap()
    out_d = nc.dram_tensor("out", [ntok, d], f32, kind="ExternalOutput").ap()
    io = (x_d, wq_d, wk_d, wv_d, wo_d, lnw_d, lnb_d, out_d)
    with tile.TileContext(nc) as tc:
        with ExitStack() as ctx:
            emit_body(ctx, tc, io, ntok, d, nh, hd, repeat=repeat)
    nc.compile()
    return nc


_CACHE = {}


def _make_runner(nc, n_cores):
    """Build a reusable sharded PJRT callable for the compiled Bass module.

    Mirrors concourse.bass2jax.run_bass_via_pjrt's multi-core path, but holds
    the jitted function so repeated kernel() calls (and timing loops) do not
    re-trace/re-compile the XLA wrapper.
    """
    import jax
    from jax.sharding import Mesh, PartitionSpec
    from jax.experimental.shard_map import shard_map
    from concourse.bass2jax import (_bass_exec_p, install_neuronx_cc_hook,
                                    partition_id_tensor)

    install_neuronx_cc_hook()
    partition_name = (nc.partition_id_tensor.name
                      if nc.partition_id_tensor else None)

    in_names, out_names, out_avals = [], [], []
    for alloc in nc.m.functions[0].allocations:
        if not isinstance(alloc, mybir.MemoryLocationSet):
            continue
        name = alloc.memorylocations[0].name
        if alloc.kind == "ExternalInput":
            if name != partition_name:
                in_names.append(name)
        elif alloc.kind == "ExternalOutput":
            out_names.append(name)
            out_avals.append(jax.core.ShapedArray(
                tuple(alloc.tensor_shape), mybir.dt.np(alloc.dtype)))
    all_names = in_names + out_names
    if partition_name is not None:
        all_names = all_names + [partition_name]

    def _body(*args):
        operands = list(args)
        if partition_name is not None:
            operands.append(partition_id_tensor())
        outs = _bass_exec_p.bind(
            *operands,
            out_avals=tuple(out_avals),
            in_names=tuple(all_names),
            out_names=tuple(out_names),
            lowering_input_output_aliases=(),
            sim_require_finite=True,
            sim_require_nnan=True,
            nc=nc,
        )
        return tuple(outs)

    devices = jax.devices()[:n_cores]
    assert len(devices) == n_cores
    mesh = Mesh(np.asarray(devices), ("core",))
    nio = len(in_names) + len(out_names)
    sharded = jax.jit(
        shard_map(_body, mesh=mesh,
                  in_specs=(PartitionSpec("core"),) * nio,
                  out_specs=(PartitionSpec("core"),) * len(out_names),
                  check_rep=False),
        keep_unused=True)
    return sharded, in_names, out_names, out_avals


def _concat_inputs(in_maps, in_names, out_avals, n_cores):
    concat_in = [np.concatenate([np.asarray(in_maps[c][name])
                                 for c in range(n_cores)], axis=0)
                 for name in in_names]
    concat_zeros = [np.zeros((n_cores * a.shape[0], *a.shape[1:]), a.dtype)
                    for a in out_avals]
    return concat_in + concat_zeros


def _run_spmd(in_maps, n_cores):
    sharded, in_names, out_names, out_avals = _CACHE["runner"]
    args = _concat_inputs(in_maps, in_names, out_avals, n_cores)
    _CACHE["last_args"] = args
    out_arrs = sharded(*args)
    return [
        {name: np.asarray(out_arrs[i]).reshape(n_cores, *out_avals[i].shape)[c]
         for i, name in enumerate(out_names)}
        for c in range(n_cores)
    ]


def kernel(x, ln_w, ln_b, w_qkv, w_out):
    x = np.asarray(x, dtype=np.float32)
    ln_w = np.asarray(ln_w, dtype=np.float32)
    ln_b = np.asarray(ln_b, dtype=np.float32)
    w_qkv = np.asarray(w_qkv, dtype=np.float32)
    w_out = np.asarray(w_out, dtype=np.float32)

    B, ntok, d = x.shape               # 4, 2048, 1024
    inner = w_out.shape[0]             # 1024
    hd = 64
    H = inner // hd                    # 16
    n_cores = 8
    gpb = n_cores // B                 # head-groups per batch (2)
    nh = H // gpb                      # heads per core (8)
    cc = nh * hd                       # 512

    if "nc" not in _CACHE:
        _CACHE["nc"] = build_nc(ntok=ntok, d=d, nh=nh, hd=hd, n_cores=n_cores)
    nc = _CACHE["nc"]

    bf = ml_dtypes.bfloat16
    wq_f = w_qkv[:, 0 * inner:1 * inner]
    wk_f = w_qkv[:, 1 * inner:2 * inner]
    wv_f = w_qkv[:, 2 * inner:3 * inner]

    in_maps = []
    for c in range(n_cores):
        b, g = divmod(c, gpb)
        cols = slice(g * cc, (g + 1) * cc)
        in_maps.append({
            "x": np.ascontiguousarray(x[b]),
            "wq": np.ascontiguousarray(wq_f[:, cols]).astype(bf),
            "wk": np.ascontiguousarray(wk_f[:, cols]).astype(bf),
            "wv": np.ascontiguousarray(wv_f[:, cols]).astype(bf),
            "wo": np.ascontiguousarray(w_out[cols, :]).astype(bf),
            "lnw": ln_w,
            "lnb": ln_b,
        })

    if "runner" not in _CACHE:
        _CACHE["runner"] = _make_runner(nc, n_cores)
    results = _run_spmd(in_maps, n_cores)
    parts = [results[c]["out"] for c in range(n_cores)]
    out = np.stack([sum(parts[b * gpb + g] for g in range(gpb))
                    for b in range(B)])
    return out.astype(np.float32)


# revision 28
# speedup vs baseline: 107.5364x; 1.0334x over previous
"""Trainium2 Bass kernel for LayerNorm + multi-head attention + out-projection.

Reference computation (f32):
    h = LayerNorm(x) * ln_w + ln_b
    q, k, v = split(h @ w_qkv)          # 16 heads, head_dim 64
    out = softmax(q k^T / 8) v          # per head, full 2048-seq attention
    return concat_heads(out) @ w_out

Sharding over 8 NeuronCores: core c -> (batch b = c // 2, head-group g = c % 2).
Each core handles one batch and 8 of the 16 heads (tensor parallel on heads:
w_qkv column-split, w_out row-split).  Each core emits a partial [2048, 1024]
output; the host sums the two partials of each batch (the all-reduce is a
2-element sum done on host after gather).

Device-side dataflow per core (all matmuls out = lhsT.T @ rhs):
  - LayerNorm in token-major layout (bn_stats/bn_aggr, per-partition scalars),
    fused per 128-token tile with PE-transpose h -> hT [d-part, tokens] (bf16)
    and the V projection; kT/qT projections emitted per 512-token block so
    they overlap the next tiles' LayerNorm (float32r so the S matmul runs the
    PE's full-rate reduced-precision fp32 path).
  - V = hT.T @ Wv in natural [tokens, cols] layout with an extra ones column
    per head (accumulates the softmax denominator for free during AV).
  - per (q-block, head-pair): S^T = kT_h.T @ qT_h into a 2-bank PSUM tile
    (two k-tiles wide) -> one 1024-wide exp on ScalarE (1/8 scale fused; S ~
    N(0,1), so no max subtraction is needed for fp32/bf16 range) ->
    O' = [V_h|1].T @ P^T accumulated over k tiles: rows 0..63 = unnormalized
    attention out (transposed), row 64 = denominator.  Head pairs sit at PSUM
    partitions 0/64 so their K=64 S-matmuls can overlap on PE row-groups.
    One PSUM->SBUF copy frees the accumulator, then normalize with DVE using
    a DMA-broadcast reciprocal into oT (bf16).
  - out = oT.T @ Wout (natural layout, streamed to DRAM per q-block).

Engine budget per core (cost model): PE ~352 us, ACT ~268 us (exp-bound
attention), DVE ~228 us; e2e estimate ~475 us.
"""

from contextlib import ExitStack

import numpy as np

import concourse.bass as bass
import concourse.tile as tile
from concourse import bacc, mybir
from concourse.masks import make_identity

import ml_dtypes

P = 128
EPS = 1e-5


def _bcast_partition(ap, n, skip_partition=True):
    """AP that reads a [1, F] access pattern broadcast to [n, F] partitions.

    skip_partition: drop the existing (size-1) partition dim of an on-chip AP;
    False for DRAM APs, whose dims are all kept as free dims.
    """
    dims = list(ap.ap[1:]) if skip_partition else list(ap.ap)
    if skip_partition:
        # SBUF source: partition step 0 is illegal, so read [1, n, F] from the
        # single source partition (step-0 repeat in a free dim) and let the
        # DMA scatter across destination partitions.
        part = list(ap.ap[0])
        return bass.AP(tensor=ap.tensor, offset=ap.offset,
                       ap=[[part[0], 1], [0, n]] + dims)
    return bass.AP(tensor=ap.tensor, offset=ap.offset, ap=[[0, n]] + dims)


def emit_body(ctx, tc, io, ntok, d, nh, hd, repeat=1):
    nc = tc.nc
    f32 = mybir.dt.float32
    bf16 = mybir.dt.bfloat16
    f32r = mybir.dt.float32r
    Act = mybir.ActivationFunctionType
    Alu = mybir.AluOpType

    cc = nh * hd            # head cols per core (512)
    n_dt = d // P           # d-model tiles (8)
    n_tt = ntok // P        # token tiles (16)
    FQ = min(512, ntok)     # q block / matmul moving size
    n_qb = ntok // FQ       # q blocks (4)
    FN = min(512, d)        # out-proj free block
    n_nb = d // FN          # out-proj col blocks (2)
    n_ct = cc // P          # head-pair tiles (4)
    bn_ch = min(512, d)     # bn_stats chunk size
    n_ch = d // bn_ch       # bn_stats chunks (2)
    vw = hd + 1             # V cols per head incl. ones column (65)
    scale = float(hd) ** -0.5

    x_d, wq_d, wk_d, wv_d, wo_d, bq_d, bk_d, bv_d, out_d = io

    # ---------------- constants & weights ----------------
    const = ctx.enter_context(tc.tile_pool(name="const", bufs=1))
    ident = const.tile([P, P], f32)
    make_identity(nc, ident[:])
    eps_sb = const.tile([P, 1], f32)
    nc.vector.memset(eps_sb[:], EPS)
    ones_row = const.tile([1, hd], f32)
    nc.vector.memset(ones_row[:], 1.0)
    # ln_b @ W biases (ln_w is folded into the weights host-side)
    bq_sb = [const.tile([P, 1], f32, tag=f"bq{j}", name=f"bq{j}")
             for j in range(n_ct)]
    bk_sb = [const.tile([P, 1], f32, tag=f"bk{j}", name=f"bk{j}")
             for j in range(n_ct)]
    for j in range(n_ct):
        nc.gpsimd.dma_start(out=bq_sb[j][:], in_=bq_d[j * P:(j + 1) * P])
        nc.gpsimd.dma_start(out=bk_sb[j][:], in_=bk_d[j * P:(j + 1) * P])
    bv_bc = const.tile([P, cc], f32)
    nc.gpsimd.dma_start(out=bv_bc[:],
                        in_=_bcast_partition(bv_d, P, skip_partition=False))
    # warm the ACT Sqrt table while the first DMAs run
    warm = const.tile([P, 1], f32)
    nc.scalar.activation(warm[:], eps_sb[:], Act.Sqrt, bias=eps_sb[:], scale=1.0)

    wpool = ctx.enter_context(tc.tile_pool(name="weights", bufs=1))
    wq_sb = [wpool.tile([P, cc], bf16, tag=f"wq{k}", name=f"wq{k}") for k in range(n_dt)]
    wk_sb = [wpool.tile([P, cc], bf16, tag=f"wk{k}", name=f"wk{k}") for k in range(n_dt)]
    wv_sb = [wpool.tile([P, cc], bf16, tag=f"wv{k}", name=f"wv{k}") for k in range(n_dt)]
    wo_sb = [wpool.tile([P, d], bf16, tag=f"wo{j}", name=f"wo{j}") for j in range(n_ct)]
    for k in range(n_dt):
        nc.gpsimd.dma_start(out=wq_sb[k][:], in_=wq_d[k * P:(k + 1) * P, :])
        nc.gpsimd.dma_start(out=wk_sb[k][:], in_=wk_d[k * P:(k + 1) * P, :])
        nc.gpsimd.dma_start(out=wv_sb[k][:], in_=wv_d[k * P:(k + 1) * P, :])
    for j in range(n_ct):
        nc.gpsimd.dma_start(out=wo_sb[j][:], in_=wo_d[j * P:(j + 1) * P, :])

    # ---------------- persistent activations ----------------
    big = ctx.enter_context(tc.tile_pool(name="big", bufs=1))
    hT = big.tile([P, n_dt, ntok], bf16, tag="hT", name="hT")
    qT = [big.tile([P, ntok], f32r, tag=f"qT{j}", name=f"qT{j}") for j in range(n_ct)]
    kT = [big.tile([P, ntok], f32r, tag=f"kT{j}", name=f"kT{j}") for j in range(n_ct)]
    V = [big.tile([P, nh * vw], bf16, tag=f"V{t}", name=f"V{t}") for t in range(n_tt)]
    oT = [big.tile([P, ntok], bf16, tag=f"oT{j}", name=f"oT{j}") for j in range(n_ct)]

    for _rep in range(repeat):
        _emit_phases(tc, locals())


def _emit_phases(tc, env):
    nc = tc.nc
    f32 = env["f32"]; bf16 = env["bf16"]; f32r = env["f32r"]
    Act = env["Act"]; Alu = env["Alu"]
    ntok = env["ntok"]; d = env["d"]; nh = env["nh"]; hd = env["hd"]
    cc = env["cc"]; n_dt = env["n_dt"]; n_tt = env["n_tt"]
    FQ = env["FQ"]; n_qb = env["n_qb"]; FN = env["FN"]; n_nb = env["n_nb"]
    n_ct = env["n_ct"]; bn_ch = env["bn_ch"]; n_ch = env["n_ch"]
    vw = env["vw"]; scale = env["scale"]
    x_d = env["x_d"]; out_d = env["out_d"]
    eps_sb = env["eps_sb"]; ident = env["ident"]
    bq_sb = env["bq_sb"]; bk_sb = env["bk_sb"]; bv_bc = env["bv_bc"]
    ones_row = env["ones_row"]
    wq_sb = env["wq_sb"]; wk_sb = env["wk_sb"]; wv_sb = env["wv_sb"]
    wo_sb = env["wo_sb"]
    hT = env["hT"]; qT = env["qT"]; kT = env["kT"]; V = env["V"]; oT = env["oT"]

    # -------- phase 1: LayerNorm + transpose + V projection (per t) --------
    with tc.tile_pool(name="xin", bufs=3) as xin_p, \
         tc.tile_pool(name="hnat", bufs=3) as h_p, \
         tc.tile_pool(name="stats", bufs=6) as st_p, \
         tc.tile_pool(name="ptr", bufs=2, space="PSUM") as ptr_p, \
         tc.tile_pool(name="psq", bufs=3, space="PSUM") as psq_p:
        for t in range(n_tt):
            xt = xin_p.tile([P, d], f32, tag="xt")
            nc.sync.dma_start(out=xt[:], in_=x_d[t * P:(t + 1) * P, :])
            st = st_p.tile([P, n_ch, 6], f32, tag="st")
            for c in range(n_ch):
                nc.vector.bn_stats(st[:, c, :], xt[:, c * bn_ch:(c + 1) * bn_ch])
            mv = st_p.tile([P, 2], f32, tag="mv")
            nc.vector.bn_aggr(mv[:], st[:])
            rstd = st_p.tile([P, 1], f32, tag="rstd")
            nc.scalar.activation(rstd[:], mv[:, 1:2], Act.Sqrt,
                                 bias=eps_sb[:], scale=1.0)
            nc.vector.reciprocal(rstd[:], rstd[:])
            ht = h_p.tile([P, d], f32, tag="ht")
            nc.vector.tensor_scalar(out=ht[:], in0=xt[:],
                                    scalar1=mv[:, 0:1], scalar2=rstd[:],
                                    op0=Alu.subtract, op1=Alu.mult)
            for g in range(0, n_dt, 4):
                ng = min(4, n_dt - g)
                ps = ptr_p.tile([P, 512], f32, tag="ptr")
                for jj in range(ng):
                    nc.tensor.transpose(ps[:, jj * P:(jj + 1) * P],
                                        ht[:, (g + jj) * P:(g + jj + 1) * P],
                                        ident[:])
                nc.vector.tensor_copy(
                    out=hT[:, g:g + ng, t * P:(t + 1) * P],
                    in_=ps[:].rearrange("p (g q) -> p g q", q=P)[:, 0:ng, :])
            # V projection for this token tile (hT for all k just landed)
            vv = V[t][:].rearrange("p (h c) -> p h c", c=vw)
            nc.vector.memset(vv[:, :, hd:hd + 1], 1.0)
            psv = psq_p.tile([P, cc], f32, tag="psq")
            for k in range(n_dt):
                nc.tensor.matmul(psv[:], lhsT=hT[:, k, t * P:(t + 1) * P],
                                 rhs=wv_sb[k][:],
                                 start=(k == 0), stop=(k == n_dt - 1))
            nc.vector.tensor_add(vv[:, :, 0:hd],
                                 psv[:].rearrange("p (h c) -> p h c", c=hd),
                                 bv_bc[:].rearrange("p (h c) -> p h c", c=hd))

            # kT / qT for each completed 512-token block (overlaps next LN)
            if (t + 1) * P % FQ == 0:
                tb = ((t + 1) * P - FQ) // FQ
                for dst, w_sb, b_sb in ((kT, wk_sb, bk_sb), (qT, wq_sb, bq_sb)):
                    for j in range(n_ct):
                        ps = psq_p.tile([P, FQ], f32, tag="psq")
                        for k in range(n_dt):
                            nc.tensor.matmul(
                                ps[:], lhsT=w_sb[k][:, j * P:(j + 1) * P],
                                rhs=hT[:, k, tb * FQ:(tb + 1) * FQ],
                                start=(k == 0), stop=(k == n_dt - 1))
                        nc.vector.tensor_scalar_add(
                            out=dst[j][:, tb * FQ:(tb + 1) * FQ], in0=ps[:],
                            scalar1=b_sb[j][:, 0:1])

    # -------- phase 3+4: attention (head pairs, 1024-wide exp) + out-proj ----
    kt_pair = 2 if n_tt % 2 == 0 else 1
    with tc.tile_pool(name="pss", bufs=2, space="PSUM") as pss_p, \
         tc.tile_pool(name="pso", bufs=2, space="PSUM") as pso_p, \
         tc.tile_pool(name="psout", bufs=2, space="PSUM") as psout_p, \
         tc.tile_pool(name="expp", bufs=4) as exp_p, \
         tc.tile_pool(name="rsp", bufs=3) as rs_p, \
         tc.tile_pool(name="outp", bufs=3) as out_p:
        def outproj(qb):
            for tt in range(qb * FQ // P, (qb + 1) * FQ // P):
                for nb in range(n_nb):
                    ps = psout_p.tile([P, FN], f32, tag="pso2")
                    for j2 in range(n_ct):
                        nc.tensor.matmul(ps[:], lhsT=oT[j2][:, tt * P:(tt + 1) * P],
                                         rhs=wo_sb[j2][:, nb * FN:(nb + 1) * FN],
                                         start=(j2 == 0), stop=(j2 == n_ct - 1))
                    ot = out_p.tile([P, FN], f32, tag="ot")
                    nc.vector.tensor_copy(ot[:], ps[:])
                    nc.sync.dma_start(
                        out=out_d[tt * P:(tt + 1) * P, nb * FN:(nb + 1) * FN],
                        in_=ot[:])

        for qb in range(n_qb):
            for j in range(n_ct):
                if j == 1 and qb > 0:
                    outproj(qb - 1)   # overlaps this q-block's remaining pairs
                pos = [pso_p.tile([vw, FQ], f32, tag="po", name=f"po{hh}")
                       for hh in range(2)]
                for kt2 in range(n_tt // kt_pair):
                    exs = []
                    for hh in range(2):
                        off = hh * hd
                        ps = pss_p.tile([P, kt_pair * FQ], f32, tag="pss")
                        for u in range(kt_pair):
                            kt = kt_pair * kt2 + u
                            nc.tensor.matmul(
                                ps[:, u * FQ:(u + 1) * FQ],
                                lhsT=kT[j][off:off + hd, kt * P:(kt + 1) * P],
                                rhs=qT[j][off:off + hd, qb * FQ:(qb + 1) * FQ],
                                start=True, stop=True)
                        ex = exp_p.tile([P, kt_pair * FQ], bf16, tag="ex")
                        nc.scalar.activation(ex[:], ps[:], Act.Exp, scale=scale)
                        exs.append(ex)
                    for hh in range(2):
                        h = 2 * j + hh
                        for u in range(kt_pair):
                            kt = kt_pair * kt2 + u
                            nc.tensor.matmul(
                                pos[hh][:],
                                lhsT=V[kt][:, h * vw:(h + 1) * vw],
                                rhs=exs[hh][:, u * FQ:(u + 1) * FQ],
                                start=(kt == 0), stop=(kt == n_tt - 1))
                for hh in range(2):
                    h = 2 * j + hh
                    off = hh * hd
                    posb = rs_p.tile([vw, FQ], f32, tag="posb")
                    nc.vector.tensor_copy(posb[:], pos[hh][:])
                    rs = rs_p.tile([1, FQ], f32, tag="rs")
                    nc.vector.reciprocal(rs[:], posb[hd:hd + 1, :])
                    rsb = rs_p.tile([hd, FQ], f32, tag="rsb")
                    nc.sync.dma_start(out=rsb[:], in_=_bcast_partition(rs[:], hd))
                    nc.vector.tensor_mul(
                        oT[j][off:off + hd, qb * FQ:(qb + 1) * FQ],
                        posb[0:hd, :], rsb[:])
        outproj(n_qb - 1)


def build_nc(ntok=2048, d=1024, nh=8, hd=64, n_cores=8, repeat=1):
    nc = bacc.Bacc("TRN2", target_bir_lowering=False, debug=False,
                   num_devices=n_cores)
    f32 = mybir.dt.float32
    bf16 = mybir.dt.bfloat16
    cc = nh * hd
    x_d = nc.dram_tensor("x", [ntok, d], f32, kind="ExternalInput").ap()
    wq_d = nc.dram_tensor("wq", [d, cc], bf16, kind="ExternalInput").ap()
    wk_d = nc.dram_tensor("wk", [d, cc], bf16, kind="ExternalInput").ap()
    wv_d = nc.dram_tensor("wv", [d, cc], bf16, kind="ExternalInput").ap()
    wo_d = nc.dram_tensor("wo", [cc, d], bf16, kind="ExternalInput").ap()
    bq_d = nc.dram_tensor("bq", [cc], f32, kind="ExternalInput").ap()
    bk_d = nc.dram_tensor("bk", [cc], f32, kind="ExternalInput").ap()
    bv_d = nc.dram_tensor("bv", [cc], f32, kind="ExternalInput").ap()
    out_d = nc.dram_tensor("out", [ntok, d], f32, kind="ExternalOutput").ap()
    io = (x_d, wq_d, wk_d, wv_d, wo_d, bq_d, bk_d, bv_d, out_d)
    with tile.TileContext(nc) as tc:
        with ExitStack() as ctx:
            emit_body(ctx, tc, io, ntok, d, nh, hd, repeat=repeat)
    nc.compile()
    return nc


_CACHE = {}


def _make_runner(nc, n_cores):
    """Build a reusable sharded PJRT callable for the compiled Bass module.

    Mirrors concourse.bass2jax.run_bass_via_pjrt's multi-core path, but holds
    the jitted function so repeated kernel() calls (and timing loops) do not
    re-trace/re-compile the XLA wrapper.
    """
    import jax
    from jax.sharding import Mesh, PartitionSpec
    from jax.experimental.shard_map import shard_map
    from concourse.bass2jax import (_bass_exec_p, install_neuronx_cc_hook,
                                    partition_id_tensor)

    install_neuronx_cc_hook()
    partition_name = (nc.partition_id_tensor.name
                      if nc.partition_id_tensor else None)

    in_names, out_names, out_avals = [], [], []
    for alloc in nc.m.functions[0].allocations:
        if not isinstance(alloc, mybir.MemoryLocationSet):
            continue
        name = alloc.memorylocations[0].name
        if alloc.kind == "ExternalInput":
            if name != partition_name:
                in_names.append(name)
        elif alloc.kind == "ExternalOutput":
            out_names.append(name)
            out_avals.append(jax.core.ShapedArray(
                tuple(alloc.tensor_shape), mybir.dt.np(alloc.dtype)))
    all_names = in_names + out_names
    if partition_name is not None:
        all_names = all_names + [partition_name]

    def _body(*args):
        operands = list(args)
        if partition_name is not None:
            operands.append(partition_id_tensor())
        outs = _bass_exec_p.bind(
            *operands,
            out_avals=tuple(out_avals),
            in_names=tuple(all_names),
            out_names=tuple(out_names),
            lowering_input_output_aliases=(),
            sim_require_finite=True,
            sim_require_nnan=True,
            nc=nc,
        )
        return tuple(outs)

    devices = jax.devices()[:n_cores]
    assert len(devices) == n_cores
    mesh = Mesh(np.asarray(devices), ("core",))
    nio = len(in_names) + len(out_names)
    sharded = jax.jit(
        shard_map(_body, mesh=mesh,
                  in_specs=(PartitionSpec("core"),) * nio,
                  out_specs=(PartitionSpec("core"),) * len(out_names),
                  check_rep=False),
        keep_unused=True)
    return sharded, in_names, out_names, out_avals


def _concat_inputs(in_maps, in_names, out_avals, n_cores):
    concat_in = [np.concatenate([np.asarray(in_maps[c][name])
                                 for c in range(n_cores)], axis=0)
                 for name in in_names]
    concat_zeros = [np.zeros((n_cores * a.shape[0], *a.shape[1:]), a.dtype)
                    for a in out_avals]
    return concat_in + concat_zeros


def _run_spmd(in_maps, n_cores):
    sharded, in_names, out_names, out_avals = _CACHE["runner"]
    args = _concat_inputs(in_maps, in_names, out_avals, n_cores)
    _CACHE["last_args"] = args
    out_arrs = sharded(*args)
    return [
        {name: np.asarray(out_arrs[i]).reshape(n_cores, *out_avals[i].shape)[c]
         for i, name in enumerate(out_names)}
        for c in range(n_cores)
    ]


def kernel(x, ln_w, ln_b, w_qkv, w_out):
    x = np.asarray(x, dtype=np.float32)
    ln_w = np.asarray(ln_w, dtype=np.float32)
    ln_b = np.asarray(ln_b, dtype=np.float32)
    w_qkv = np.asarray(w_qkv, dtype=np.float32)
    w_out = np.asarray(w_out, dtype=np.float32)

    B, ntok, d = x.shape               # 4, 2048, 1024
    inner = w_out.shape[0]             # 1024
    hd = 64
    H = inner // hd                    # 16
    n_cores = 8
    gpb = n_cores // B                 # head-groups per batch (2)
    nh = H // gpb                      # heads per core (8)
    cc = nh * hd                       # 512

    if "nc" not in _CACHE:
        _CACHE["nc"] = build_nc(ntok=ntok, d=d, nh=nh, hd=hd, n_cores=n_cores)
    nc = _CACHE["nc"]

    bf = ml_dtypes.bfloat16
    # fold the LayerNorm affine into the projections (exact):
    #   h = (x - mu) * rstd * ln_w + ln_b
    #   h @ W = ((x - mu) * rstd) @ (diag(ln_w) W) + (ln_b @ W)
    wq_f = ln_w[:, None] * w_qkv[:, 0 * inner:1 * inner]
    wk_f = ln_w[:, None] * w_qkv[:, 1 * inner:2 * inner]
    wv_f = ln_w[:, None] * w_qkv[:, 2 * inner:3 * inner]
    bq_f = ln_b @ w_qkv[:, 0 * inner:1 * inner]
    bk_f = ln_b @ w_qkv[:, 1 * inner:2 * inner]
    bv_f = ln_b @ w_qkv[:, 2 * inner:3 * inner]

    in_maps = []
    for c in range(n_cores):
        b, g = divmod(c, gpb)
        cols = slice(g * cc, (g + 1) * cc)
        in_maps.append({
            "x": np.ascontiguousarray(x[b]),
            "wq": np.ascontiguousarray(wq_f[:, cols]).astype(bf),
            "wk": np.ascontiguousarray(wk_f[:, cols]).astype(bf),
            "wv": np.ascontiguousarray(wv_f[:, cols]).astype(bf),
            "wo": np.ascontiguousarray(w_out[cols, :]).astype(bf),
            "bq": np.ascontiguousarray(bq_f[cols]).astype(np.float32),
            "bk": np.ascontiguousarray(bk_f[cols]).astype(np.float32),
            "bv": np.ascontiguousarray(bv_f[cols]).astype(np.float32),
        })

    if "runner" not in _CACHE:
        _CACHE["runner"] = _make_runner(nc, n_cores)
    results = _run_spmd(in_maps, n_cores)
    parts = [results[c]["out"] for c in range(n_cores)]
    out = np.stack([sum(parts[b * gpb + g] for g in range(gpb))
                    for b in range(B)])
    return out.astype(np.float32)


# revision 30
# speedup vs baseline: 108.5491x; 1.0094x over previous
"""Trainium2 Bass kernel for LayerNorm + multi-head attention + out-projection.

Reference computation (f32):
    h = LayerNorm(x) * ln_w + ln_b
    q, k, v = split(h @ w_qkv)          # 16 heads, head_dim 64
    out = softmax(q k^T / 8) v          # per head, full 2048-seq attention
    return concat_heads(out) @ w_out

Sharding over 8 NeuronCores: core c -> (batch b = c // 2, head-group g = c % 2).
Each core handles one batch and 8 of the 16 heads (tensor parallel on heads:
w_qkv column-split, w_out row-split).  Each core emits a partial [2048, 1024]
output; the host sums the two partials of each batch (the all-reduce is a
2-element sum done on host after gather).

Device-side dataflow per core (all matmuls out = lhsT.T @ rhs):
  - LayerNorm in token-major layout (bn_stats/bn_aggr, per-partition scalars),
    fused per 128-token tile with PE-transpose h -> hT [d-part, tokens] (bf16)
    and the V projection; kT/qT projections emitted per 512-token block so
    they overlap the next tiles' LayerNorm (float32r so the S matmul runs the
    PE's full-rate reduced-precision fp32 path).
  - V = hT.T @ Wv in natural [tokens, cols] layout with an extra ones column
    per head (accumulates the softmax denominator for free during AV).
  - per (q-block, head-pair): S^T = kT_h.T @ qT_h into a 2-bank PSUM tile
    (two k-tiles wide) -> one 1024-wide exp on ScalarE (1/8 scale fused; S ~
    N(0,1), so no max subtraction is needed for fp32/bf16 range) ->
    O' = [V_h|1].T @ P^T accumulated over k tiles: rows 0..63 = unnormalized
    attention out (transposed), row 64 = denominator.  Head pairs sit at PSUM
    partitions 0/64 so their K=64 S-matmuls can overlap on PE row-groups.
    One PSUM->SBUF copy frees the accumulator, then normalize with DVE using
    a DMA-broadcast reciprocal into oT (bf16).
  - out = oT.T @ Wout (natural layout, streamed to DRAM per q-block).

The LayerNorm affine is folded into the projections host-side (exact):
h @ W = ((x - mu) * rstd) @ (diag(ln_w) W) + ln_b @ W, so the device only
computes (x - mu) * rstd and adds the ln_b @ W bias during the PSUM->SBUF
copy of each projection.  Out-projections are emitted one head-pair into the
next q-block so ScalarE keeps exp-ing while PE projects.

Engine budget per core (cost model): PE ~352 us, ACT ~268 us (exp-bound
attention), DVE ~166 us; e2e estimate ~460 us.
"""

from contextlib import ExitStack

import numpy as np

import concourse.bass as bass
import concourse.tile as tile
from concourse import bacc, mybir
from concourse.masks import make_identity

import ml_dtypes

P = 128
EPS = 1e-5


def _bcast_partition(ap, n, skip_partition=True):
    """AP that reads a [1, F] access pattern broadcast to [n, F] partitions.

    skip_partition: drop the existing (size-1) partition dim of an on-chip AP;
    False for DRAM APs, whose dims are all kept as free dims.
    """
    dims = list(ap.ap[1:]) if skip_partition else list(ap.ap)
    if skip_partition:
        # SBUF source: partition step 0 is illegal, so read [1, n, F] from the
        # single source partition (step-0 repeat in a free dim) and let the
        # DMA scatter across destination partitions.
        part = list(ap.ap[0])
        return bass.AP(tensor=ap.tensor, offset=ap.offset,
                       ap=[[part[0], 1], [0, n]] + dims)
    return bass.AP(tensor=ap.tensor, offset=ap.offset, ap=[[0, n]] + dims)


def emit_body(ctx, tc, io, ntok, d, nh, hd, repeat=1):
    nc = tc.nc
    f32 = mybir.dt.float32
    bf16 = mybir.dt.bfloat16
    f32r = mybir.dt.float32r
    Act = mybir.ActivationFunctionType
    Alu = mybir.AluOpType

    cc = nh * hd            # head cols per core (512)
    n_dt = d // P           # d-model tiles (8)
    n_tt = ntok // P        # token tiles (16)
    FQ = min(512, ntok)     # q block / matmul moving size
    n_qb = ntok // FQ       # q blocks (4)
    FN = min(512, d)        # out-proj free block
    n_nb = d // FN          # out-proj col blocks (2)
    n_ct = cc // P          # head-pair tiles (4)
    bn_ch = min(512, d)     # bn_stats chunk size
    n_ch = d // bn_ch       # bn_stats chunks (2)
    vw = hd + 1             # V cols per head incl. ones column (65)
    scale = float(hd) ** -0.5

    x_d, wq_d, wk_d, wv_d, wo_d, bq_d, bk_d, bv_d, out_d = io

    # ---------------- constants & weights ----------------
    const = ctx.enter_context(tc.tile_pool(name="const", bufs=1))
    ident = const.tile([P, P], f32)
    make_identity(nc, ident[:])
    eps_sb = const.tile([P, 1], f32)
    nc.vector.memset(eps_sb[:], EPS)
    ones_row = const.tile([1, hd], f32)
    nc.vector.memset(ones_row[:], 1.0)
    # ln_b @ W biases (ln_w is folded into the weights host-side)
    bq_sb = [const.tile([P, 1], f32, tag=f"bq{j}", name=f"bq{j}")
             for j in range(n_ct)]
    bk_sb = [const.tile([P, 1], f32, tag=f"bk{j}", name=f"bk{j}")
             for j in range(n_ct)]
    for j in range(n_ct):
        nc.gpsimd.dma_start(out=bq_sb[j][:], in_=bq_d[j * P:(j + 1) * P])
        nc.gpsimd.dma_start(out=bk_sb[j][:], in_=bk_d[j * P:(j + 1) * P])
    bv_bc = const.tile([P, cc], f32)
    nc.gpsimd.dma_start(out=bv_bc[:],
                        in_=_bcast_partition(bv_d, P, skip_partition=False))
    # warm the ACT Sqrt table while the first DMAs run
    warm = const.tile([P, 1], f32)
    nc.scalar.activation(warm[:], eps_sb[:], Act.Sqrt, bias=eps_sb[:], scale=1.0)

    wpool = ctx.enter_context(tc.tile_pool(name="weights", bufs=1))
    wq_sb = [wpool.tile([P, cc], bf16, tag=f"wq{k}", name=f"wq{k}") for k in range(n_dt)]
    wk_sb = [wpool.tile([P, cc], bf16, tag=f"wk{k}", name=f"wk{k}") for k in range(n_dt)]
    wv_sb = [wpool.tile([P, cc], bf16, tag=f"wv{k}", name=f"wv{k}") for k in range(n_dt)]
    wo_sb = [wpool.tile([P, d], bf16, tag=f"wo{j}", name=f"wo{j}") for j in range(n_ct)]
    for k in range(n_dt):
        nc.gpsimd.dma_start(out=wq_sb[k][:], in_=wq_d[k * P:(k + 1) * P, :])
        nc.gpsimd.dma_start(out=wk_sb[k][:], in_=wk_d[k * P:(k + 1) * P, :])
        nc.gpsimd.dma_start(out=wv_sb[k][:], in_=wv_d[k * P:(k + 1) * P, :])
    for j in range(n_ct):
        nc.gpsimd.dma_start(out=wo_sb[j][:], in_=wo_d[j * P:(j + 1) * P, :])

    # ---------------- persistent activations ----------------
    big = ctx.enter_context(tc.tile_pool(name="big", bufs=1))
    hT = big.tile([P, n_dt, ntok], bf16, tag="hT", name="hT")
    qT = [big.tile([P, ntok], f32r, tag=f"qT{j}", name=f"qT{j}") for j in range(n_ct)]
    kT = [big.tile([P, ntok], f32r, tag=f"kT{j}", name=f"kT{j}") for j in range(n_ct)]
    V = [big.tile([P, nh * vw], bf16, tag=f"V{t}", name=f"V{t}") for t in range(n_tt)]
    oT = [big.tile([P, ntok], bf16, tag=f"oT{j}", name=f"oT{j}") for j in range(n_ct)]

    for _rep in range(repeat):
        _emit_phases(tc, locals())


def _emit_phases(tc, env):
    nc = tc.nc
    f32 = env["f32"]; bf16 = env["bf16"]; f32r = env["f32r"]
    Act = env["Act"]; Alu = env["Alu"]
    ntok = env["ntok"]; d = env["d"]; nh = env["nh"]; hd = env["hd"]
    cc = env["cc"]; n_dt = env["n_dt"]; n_tt = env["n_tt"]
    FQ = env["FQ"]; n_qb = env["n_qb"]; FN = env["FN"]; n_nb = env["n_nb"]
    n_ct = env["n_ct"]; bn_ch = env["bn_ch"]; n_ch = env["n_ch"]
    vw = env["vw"]; scale = env["scale"]
    x_d = env["x_d"]; out_d = env["out_d"]
    eps_sb = env["eps_sb"]; ident = env["ident"]
    bq_sb = env["bq_sb"]; bk_sb = env["bk_sb"]; bv_bc = env["bv_bc"]
    ones_row = env["ones_row"]
    wq_sb = env["wq_sb"]; wk_sb = env["wk_sb"]; wv_sb = env["wv_sb"]
    wo_sb = env["wo_sb"]
    hT = env["hT"]; qT = env["qT"]; kT = env["kT"]; V = env["V"]; oT = env["oT"]

    # -------- phase 1: LayerNorm + transpose + V projection (per t) --------
    with tc.tile_pool(name="xin", bufs=3) as xin_p, \
         tc.tile_pool(name="hnat", bufs=3) as h_p, \
         tc.tile_pool(name="stats", bufs=6) as st_p, \
         tc.tile_pool(name="ptr", bufs=2, space="PSUM") as ptr_p, \
         tc.tile_pool(name="psq", bufs=3, space="PSUM") as psq_p:
        for t in range(n_tt):
            xt = xin_p.tile([P, d], f32, tag="xt")
            nc.sync.dma_start(out=xt[:], in_=x_d[t * P:(t + 1) * P, :])
            st = st_p.tile([P, n_ch, 6], f32, tag="st")
            for c in range(n_ch):
                nc.vector.bn_stats(st[:, c, :], xt[:, c * bn_ch:(c + 1) * bn_ch])
            mv = st_p.tile([P, 2], f32, tag="mv")
            nc.vector.bn_aggr(mv[:], st[:])
            rstd = st_p.tile([P, 1], f32, tag="rstd")
            nc.scalar.activation(rstd[:], mv[:, 1:2], Act.Sqrt,
                                 bias=eps_sb[:], scale=1.0)
            nc.vector.reciprocal(rstd[:], rstd[:])
            ht = h_p.tile([P, d], f32, tag="ht")
            half = d // 2
            for c2 in range(2):
                nc.vector.tensor_scalar(out=ht[:, c2 * half:(c2 + 1) * half],
                                        in0=xt[:, c2 * half:(c2 + 1) * half],
                                        scalar1=mv[:, 0:1], scalar2=rstd[:],
                                        op0=Alu.subtract, op1=Alu.mult)
            for g in range(0, n_dt, 4):
                ng = min(4, n_dt - g)
                ps = ptr_p.tile([P, 512], f32, tag="ptr")
                for jj in range(ng):
                    nc.tensor.transpose(ps[:, jj * P:(jj + 1) * P],
                                        ht[:, (g + jj) * P:(g + jj + 1) * P],
                                        ident[:])
                nc.vector.tensor_copy(
                    out=hT[:, g:g + ng, t * P:(t + 1) * P],
                    in_=ps[:].rearrange("p (g q) -> p g q", q=P)[:, 0:ng, :])
            # V projection for this token tile (hT for all k just landed)
            vv = V[t][:].rearrange("p (h c) -> p h c", c=vw)
            nc.vector.memset(vv[:, :, hd:hd + 1], 1.0)
            psv = psq_p.tile([P, cc], f32, tag="psq")
            for k in range(n_dt):
                nc.tensor.matmul(psv[:], lhsT=hT[:, k, t * P:(t + 1) * P],
                                 rhs=wv_sb[k][:],
                                 start=(k == 0), stop=(k == n_dt - 1))
            nc.vector.tensor_add(vv[:, :, 0:hd],
                                 psv[:].rearrange("p (h c) -> p h c", c=hd),
                                 bv_bc[:].rearrange("p (h c) -> p h c", c=hd))

            # kT / qT for each completed 512-token block (overlaps next LN)
            if (t + 1) * P % FQ == 0:
                tb = ((t + 1) * P - FQ) // FQ
                for dst, w_sb, b_sb in ((kT, wk_sb, bk_sb), (qT, wq_sb, bq_sb)):
                    for j in range(n_ct):
                        ps = psq_p.tile([P, FQ], f32, tag="psq")
                        for k in range(n_dt):
                            nc.tensor.matmul(
                                ps[:], lhsT=w_sb[k][:, j * P:(j + 1) * P],
                                rhs=hT[:, k, tb * FQ:(tb + 1) * FQ],
                                start=(k == 0), stop=(k == n_dt - 1))
                        nc.vector.tensor_scalar_add(
                            out=dst[j][:, tb * FQ:(tb + 1) * FQ], in0=ps[:],
                            scalar1=b_sb[j][:, 0:1])

    # -------- phase 3+4: attention (head pairs, 1024-wide exp) + out-proj ----
    kt_pair = 2 if n_tt % 2 == 0 else 1
    with tc.tile_pool(name="pss", bufs=2, space="PSUM") as pss_p, \
         tc.tile_pool(name="pso", bufs=2, space="PSUM") as pso_p, \
         tc.tile_pool(name="psout", bufs=2, space="PSUM") as psout_p, \
         tc.tile_pool(name="expp", bufs=4) as exp_p, \
         tc.tile_pool(name="rsp", bufs=3) as rs_p, \
         tc.tile_pool(name="outp", bufs=3) as out_p:
        def outproj(qb):
            for tt in range(qb * FQ // P, (qb + 1) * FQ // P):
                for nb in range(n_nb):
                    ps = psout_p.tile([P, FN], f32, tag="pso2")
                    for j2 in range(n_ct):
                        nc.tensor.matmul(ps[:], lhsT=oT[j2][:, tt * P:(tt + 1) * P],
                                         rhs=wo_sb[j2][:, nb * FN:(nb + 1) * FN],
                                         start=(j2 == 0), stop=(j2 == n_ct - 1))
                    ot = out_p.tile([P, FN], f32, tag="ot")
                    nc.vector.tensor_copy(ot[:], ps[:])
                    nc.sync.dma_start(
                        out=out_d[tt * P:(tt + 1) * P, nb * FN:(nb + 1) * FN],
                        in_=ot[:])

        for qb in range(n_qb):
            for j in range(n_ct):
                if j == 1 and qb > 0:
                    outproj(qb - 1)   # overlaps this q-block's remaining pairs
                pos = [pso_p.tile([vw, FQ], f32, tag="po", name=f"po{hh}")
                       for hh in range(2)]
                for kt2 in range(n_tt // kt_pair):
                    exs = []
                    for hh in range(2):
                        off = hh * hd
                        ps = pss_p.tile([P, kt_pair * FQ], f32, tag="pss")
                        for u in range(kt_pair):
                            kt = kt_pair * kt2 + u
                            nc.tensor.matmul(
                                ps[:, u * FQ:(u + 1) * FQ],
                                lhsT=kT[j][off:off + hd, kt * P:(kt + 1) * P],
                                rhs=qT[j][off:off + hd, qb * FQ:(qb + 1) * FQ],
                                start=True, stop=True)
                        ex = exp_p.tile([P, kt_pair * FQ], bf16, tag="ex")
                        nc.scalar.activation(ex[:], ps[:], Act.Exp, scale=scale)
                        exs.append(ex)
                    for hh in range(2):
                        h = 2 * j + hh
                        for u in range(kt_pair):
                            kt = kt_pair * kt2 + u
                            nc.tensor.matmul(
                                pos[hh][:],
                                lhsT=V[kt][:, h * vw:(h + 1) * vw],
                                rhs=exs[hh][:, u * FQ:(u + 1) * FQ],
                                start=(kt == 0), stop=(kt == n_tt - 1))
                for hh in range(2):
                    h = 2 * j + hh
                    off = hh * hd
                    posb = rs_p.tile([vw, FQ], f32, tag="posb")
                    nc.vector.tensor_copy(posb[:], pos[hh][:])
                    rs = rs_p.tile([1, FQ], f32, tag="rs")
                    nc.vector.reciprocal(rs[:], posb[hd:hd + 1, :])
                    rsb = rs_p.tile([hd, FQ], f32, tag="rsb")
                    nc.sync.dma_start(out=rsb[:], in_=_bcast_partition(rs[:], hd))
                    nc.vector.tensor_mul(
                        oT[j][off:off + hd, qb * FQ:(qb + 1) * FQ],
                        posb[0:hd, :], rsb[:])
        outproj(n_qb - 1)


def build_nc(ntok=2048, d=1024, nh=8, hd=64, n_cores=8, repeat=1):
    nc = bacc.Bacc("TRN2", target_bir_lowering=False, debug=False,
                   num_devices=n_cores)
    f32 = mybir.dt.float32
    bf16 = mybir.dt.bfloat16
    cc = nh * hd
    x_d = nc.dram_tensor("x", [ntok, d], f32, kind="ExternalInput").ap()
    wq_d = nc.dram_tensor("wq", [d, cc], bf16, kind="ExternalInput").ap()
    wk_d = nc.dram_tensor("wk", [d, cc], bf16, kind="ExternalInput").ap()
    wv_d = nc.dram_tensor("wv", [d, cc], bf16, kind="ExternalInput").ap()
    wo_d = nc.dram_tensor("wo", [cc, d], bf16, kind="ExternalInput").ap()
    bq_d = nc.dram_tensor("bq", [cc], f32, kind="ExternalInput").ap()
    bk_d = nc.dram_tensor("bk", [cc], f32, kind="ExternalInput").ap()
    bv_d = nc.dram_tensor("bv", [cc], f32, kind="ExternalInput").ap()
    out_d = nc.dram_tensor("out", [ntok, d], f32, kind="ExternalOutput").ap()
    io = (x_d, wq_d, wk_d, wv_d, wo_d, bq_d, bk_d, bv_d, out_d)
    with tile.TileContext(nc) as tc:
        with ExitStack() as ctx:
            emit_body(ctx, tc, io, ntok, d, nh, hd, repeat=repeat)
    nc.compile()
    return nc


_CACHE = {}


def _make_runner(nc, n_cores):
    """Build a reusable sharded PJRT callable for the compiled Bass module.

    Mirrors concourse.bass2jax.run_bass_via_pjrt's multi-core path, but holds
    the jitted function so repeated kernel() calls (and timing loops) do not
    re-trace/re-compile the XLA wrapper.
    """
    import jax
    from jax.sharding import Mesh, PartitionSpec
    from jax.experimental.shard_map import shard_map
    from concourse.bass2jax import (_bass_exec_p, install_neuronx_cc_hook,
                                    partition_id_tensor)

    install_neuronx_cc_hook()
    partition_name = (nc.partition_id_tensor.name
                      if nc.partition_id_tensor else None)

    in_names, out_names, out_avals = [], [], []
    for alloc in nc.m.functions[0].allocations:
        if not isinstance(alloc, mybir.MemoryLocationSet):
            continue
        name = alloc.memorylocations[0].name
        if alloc.kind == "ExternalInput":
            if name != partition_name:
                in_names.append(name)
        elif alloc.kind == "ExternalOutput":
            out_names.append(name)
            out_avals.append(jax.core.ShapedArray(
                tuple(alloc.tensor_shape), mybir.dt.np(alloc.dtype)))
    all_names = in_names + out_names
    if partition_name is not None:
        all_names = all_names + [partition_name]

    def _body(*args):
        operands = list(args)
        if partition_name is not None:
            operands.append(partition_id_tensor())
        outs = _bass_exec_p.bind(
            *operands,
            out_avals=tuple(out_avals),
            in_names=tuple(all_names),
            out_names=tuple(out_names),
            lowering_input_output_aliases=(),
            sim_require_finite=True,
            sim_require_nnan=True,
            nc=nc,
        )
        return tuple(outs)

    devices = jax.devices()[:n_cores]
    assert len(devices) == n_cores
    mesh = Mesh(np.asarray(devices), ("core",))
    nio = len(in_names) + len(out_names)
    sharded = jax.jit(
        shard_map(_body, mesh=mesh,
                  in_specs=(PartitionSpec("core"),) * nio,
                  out_specs=(PartitionSpec("core"),) * len(out_names),
                  check_rep=False),
        keep_unused=True)
    return sharded, in_names, out_names, out_avals


def _concat_inputs(in_maps, in_names, out_avals, n_cores):
    concat_in = [np.concatenate([np.asarray(in_maps[c][name])
                                 for c in range(n_cores)], axis=0)
                 for name in in_names]
    concat_zeros = [np.zeros((n_cores * a.shape[0], *a.shape[1:]), a.dtype)
                    for a in out_avals]
    return concat_in + concat_zeros


def _run_spmd(in_maps, n_cores):
    sharded, in_names, out_names, out_avals = _CACHE["runner"]
    args = _concat_inputs(in_maps, in_names, out_avals, n_cores)
    _CACHE["last_args"] = args
    out_arrs = sharded(*args)
    return [
        {name: np.asarray(out_arrs[i]).reshape(n_cores, *out_avals[i].shape)[c]
         for i, name in enumerate(out_names)}
        for c in range(n_cores)
    ]


def kernel(x, ln_w, ln_b, w_qkv, w_out):
    x = np.asarray(x, dtype=np.float32)
    ln_w = np.asarray(ln_w, dtype=np.float32)
    ln_b = np.asarray(ln_b, dtype=np.float32)
    w_qkv = np.asarray(w_qkv, dtype=np.float32)
    w_out = np.asarray(w_out, dtype=np.float32)

    B, ntok, d = x.shape               # 4, 2048, 1024
    inner = w_out.shape[0]             # 1024
    hd = 64
    H = inner // hd                    # 16
    n_cores = 8
    gpb = n_cores // B                 # head-groups per batch (2)
    nh = H // gpb                      # heads per core (8)
    cc = nh * hd                       # 512

    if "nc" not in _CACHE:
        _CACHE["nc"] = build_nc(ntok=ntok, d=d, nh=nh, hd=hd, n_cores=n_cores)
    nc = _CACHE["nc"]

    bf = ml_dtypes.bfloat16
    # fold the LayerNorm affine into the projections (exact):
    #   h = (x - mu) * rstd * ln_w + ln_b
    #   h @ W = ((x - mu) * rstd) @ (diag(ln_w) W) + (ln_b @ W)
    wq_f = ln_w[:, None] * w_qkv[:, 0 * inner:1 * inner]
    wk_f = ln_w[:, None] * w_qkv[:, 1 * inner:2 * inner]
    wv_f = ln_w[:, None] * w_qkv[:, 2 * inner:3 * inner]
    bq_f = ln_b @ w_qkv[:, 0 * inner:1 * inner]
    bk_f = ln_b @ w_qkv[:, 1 * inner:2 * inner]
    bv_f = ln_b @ w_qkv[:, 2 * inner:3 * inner]

    in_maps = []
    for c in range(n_cores):
        b, g = divmod(c, gpb)
        cols = slice(g * cc, (g + 1) * cc)
        in_maps.append({
            "x": np.ascontiguousarray(x[b]),
            "wq": np.ascontiguousarray(wq_f[:, cols]).astype(bf),
            "wk": np.ascontiguousarray(wk_f[:, cols]).astype(bf),
            "wv": np.ascontiguousarray(wv_f[:, cols]).astype(bf),
            "wo": np.ascontiguousarray(w_out[cols, :]).astype(bf),
            "bq": np.ascontiguousarray(bq_f[cols]).astype(np.float32),
            "bk": np.ascontiguousarray(bk_f[cols]).astype(np.float32),
            "bv": np.ascontiguousarray(bv_f[cols]).astype(np.float32),
        })

    if "runner" not in _CACHE:
        _CACHE["runner"] = _make_runner(nc, n_cores)
    results = _run_spmd(in_maps, n_cores)
    parts = [results[c]["out"] for c in range(n_cores)]
    out = np.stack([sum(parts[b * gpb + g] for g in range(gpb))
                    for b in range(B)])
    return out.astype(np.float32)


# revision 32
# speedup vs baseline: 110.2747x; 1.0159x over previous
"""Trainium2 Bass kernel for LayerNorm + multi-head attention + out-projection.

Reference computation (f32):
    h = LayerNorm(x) * ln_w + ln_b
    q, k, v = split(h @ w_qkv)          # 16 heads, head_dim 64
    out = softmax(q k^T / 8) v          # per head, full 2048-seq attention
    return concat_heads(out) @ w_out

Sharding over 8 NeuronCores: core c -> (batch b = c // 2, head-group g = c % 2).
Each core handles one batch and 8 of the 16 heads (tensor parallel on heads:
w_qkv column-split, w_out row-split).  Each core emits a partial [2048, 1024]
output; the host sums the two partials of each batch (the all-reduce is a
2-element sum done on host after gather).

Device-side dataflow per core (all matmuls out = lhsT.T @ rhs):
  - LayerNorm in token-major layout (bn_stats/bn_aggr, per-partition scalars),
    fused per 128-token tile with PE-transpose h -> hT [d-part, tokens] (bf16)
    and the V projection; kT/qT projections emitted per 512-token block so
    they overlap the next tiles' LayerNorm (float32r so the S matmul runs the
    PE's full-rate reduced-precision fp32 path).
  - V = hT.T @ Wv in natural [tokens, cols] layout with an extra ones column
    per head (accumulates the softmax denominator for free during AV).
  - per (q-block, head-pair): S^T = kT_h.T @ qT_h into a 2-bank PSUM tile
    (two k-tiles wide) -> one 1024-wide exp on ScalarE (1/8 scale fused; S ~
    N(0,1), so no max subtraction is needed for fp32/bf16 range) ->
    O' = [V_h|1].T @ P^T accumulated over k tiles: rows 0..63 = unnormalized
    attention out (transposed), row 64 = denominator.  Head pairs sit at PSUM
    partitions 0/64 so their K=64 S-matmuls can overlap on PE row-groups.
    One PSUM->SBUF copy frees the accumulator, then normalize with DVE using
    a DMA-broadcast reciprocal into oT (bf16).
  - out = oT.T @ Wout (natural layout, streamed to DRAM per q-block).

The LayerNorm affine is folded into the projections host-side (exact):
h @ W = ((x - mu) * rstd) @ (diag(ln_w) W) + ln_b @ W, so the device only
computes (x - mu) * rstd and adds the ln_b @ W bias during the PSUM->SBUF
copy of each projection.  Out-projections are emitted one head-pair into the
next q-block so ScalarE keeps exp-ing while PE projects.

Engine budget per core (cost model): PE ~352 us, ACT ~268 us (exp-bound
attention), DVE ~166 us; e2e estimate ~456 us (HW-validated rel err 4.8e-3).
"""

from contextlib import ExitStack

import numpy as np

import concourse.bass as bass
import concourse.tile as tile
from concourse import bacc, mybir
from concourse.masks import make_identity

import ml_dtypes

P = 128
EPS = 1e-5


def _bcast_partition(ap, n, skip_partition=True):
    """AP that reads a [1, F] access pattern broadcast to [n, F] partitions.

    skip_partition: drop the existing (size-1) partition dim of an on-chip AP;
    False for DRAM APs, whose dims are all kept as free dims.
    """
    dims = list(ap.ap[1:]) if skip_partition else list(ap.ap)
    if skip_partition:
        # SBUF source: partition step 0 is illegal, so read [1, n, F] from the
        # single source partition (step-0 repeat in a free dim) and let the
        # DMA scatter across destination partitions.
        part = list(ap.ap[0])
        return bass.AP(tensor=ap.tensor, offset=ap.offset,
                       ap=[[part[0], 1], [0, n]] + dims)
    return bass.AP(tensor=ap.tensor, offset=ap.offset, ap=[[0, n]] + dims)


def emit_body(ctx, tc, io, ntok, d, nh, hd, repeat=1):
    nc = tc.nc
    f32 = mybir.dt.float32
    bf16 = mybir.dt.bfloat16
    f32r = mybir.dt.float32r
    Act = mybir.ActivationFunctionType
    Alu = mybir.AluOpType

    cc = nh * hd            # head cols per core (512)
    n_dt = d // P           # d-model tiles (8)
    n_tt = ntok // P        # token tiles (16)
    FQ = min(512, ntok)     # q block / matmul moving size
    n_qb = ntok // FQ       # q blocks (4)
    FN = min(512, d)        # out-proj free block
    n_nb = d // FN          # out-proj col blocks (2)
    n_ct = cc // P          # head-pair tiles (4)
    bn_ch = min(512, d)     # bn_stats chunk size
    n_ch = d // bn_ch       # bn_stats chunks (2)
    vw = hd + 1             # V cols per head incl. ones column (65)
    scale = float(hd) ** -0.5

    x_d, wq_d, wk_d, wv_d, wo_d, bq_d, bk_d, bv_d, out_d = io

    # ---------------- constants & weights ----------------
    const = ctx.enter_context(tc.tile_pool(name="const", bufs=1))
    ident = const.tile([P, P], f32)
    make_identity(nc, ident[:])
    eps_sb = const.tile([P, 1], f32)
    nc.vector.memset(eps_sb[:], EPS)
    ones_row = const.tile([1, hd], f32)
    nc.vector.memset(ones_row[:], 1.0)
    # ln_b @ W biases (ln_w is folded into the weights host-side)
    bq_sb = [const.tile([P, 1], f32, tag=f"bq{j}", name=f"bq{j}")
             for j in range(n_ct)]
    bk_sb = [const.tile([P, 1], f32, tag=f"bk{j}", name=f"bk{j}")
             for j in range(n_ct)]
    for j in range(n_ct):
        nc.gpsimd.dma_start(out=bq_sb[j][:], in_=bq_d[j * P:(j + 1) * P])
        nc.gpsimd.dma_start(out=bk_sb[j][:], in_=bk_d[j * P:(j + 1) * P])
    bv_bc = const.tile([P, cc], f32)
    nc.gpsimd.dma_start(out=bv_bc[:],
                        in_=_bcast_partition(bv_d, P, skip_partition=False))
    # warm the ACT Sqrt table while the first DMAs run
    warm = const.tile([P, 1], f32)
    nc.scalar.activation(warm[:], eps_sb[:], Act.Sqrt, bias=eps_sb[:], scale=1.0)

    wpool = ctx.enter_context(tc.tile_pool(name="weights", bufs=1))
    wq_sb = [wpool.tile([P, cc], bf16, tag=f"wq{k}", name=f"wq{k}") for k in range(n_dt)]
    wk_sb = [wpool.tile([P, cc], bf16, tag=f"wk{k}", name=f"wk{k}") for k in range(n_dt)]
    wv_sb = [wpool.tile([P, cc], bf16, tag=f"wv{k}", name=f"wv{k}") for k in range(n_dt)]
    wo_sb = [wpool.tile([P, d], bf16, tag=f"wo{j}", name=f"wo{j}") for j in range(n_ct)]
    for k in range(n_dt):
        nc.gpsimd.dma_start(out=wq_sb[k][:], in_=wq_d[k * P:(k + 1) * P, :])
        nc.gpsimd.dma_start(out=wk_sb[k][:], in_=wk_d[k * P:(k + 1) * P, :])
        nc.gpsimd.dma_start(out=wv_sb[k][:], in_=wv_d[k * P:(k + 1) * P, :])
    for j in range(n_ct):
        nc.gpsimd.dma_start(out=wo_sb[j][:], in_=wo_d[j * P:(j + 1) * P, :])

    # ---------------- persistent activations ----------------
    big = ctx.enter_context(tc.tile_pool(name="big", bufs=1))
    hT = big.tile([P, n_dt, ntok], bf16, tag="hT", name="hT")
    qT = [big.tile([P, ntok], f32r, tag=f"qT{j}", name=f"qT{j}") for j in range(n_ct)]
    kT = [big.tile([P, ntok], f32r, tag=f"kT{j}", name=f"kT{j}") for j in range(n_ct)]
    V = [big.tile([P, nh * vw], bf16, tag=f"V{t}", name=f"V{t}") for t in range(n_tt)]
    oT = [big.tile([P, ntok], bf16, tag=f"oT{j}", name=f"oT{j}") for j in range(n_ct)]

    for _rep in range(repeat):
        _emit_phases(tc, locals())


def _emit_phases(tc, env):
    nc = tc.nc
    f32 = env["f32"]; bf16 = env["bf16"]; f32r = env["f32r"]
    Act = env["Act"]; Alu = env["Alu"]
    ntok = env["ntok"]; d = env["d"]; nh = env["nh"]; hd = env["hd"]
    cc = env["cc"]; n_dt = env["n_dt"]; n_tt = env["n_tt"]
    FQ = env["FQ"]; n_qb = env["n_qb"]; FN = env["FN"]; n_nb = env["n_nb"]
    n_ct = env["n_ct"]; bn_ch = env["bn_ch"]; n_ch = env["n_ch"]
    vw = env["vw"]; scale = env["scale"]
    x_d = env["x_d"]; out_d = env["out_d"]
    eps_sb = env["eps_sb"]; ident = env["ident"]
    bq_sb = env["bq_sb"]; bk_sb = env["bk_sb"]; bv_bc = env["bv_bc"]
    ones_row = env["ones_row"]
    wq_sb = env["wq_sb"]; wk_sb = env["wk_sb"]; wv_sb = env["wv_sb"]
    wo_sb = env["wo_sb"]
    hT = env["hT"]; qT = env["qT"]; kT = env["kT"]; V = env["V"]; oT = env["oT"]

    # -------- phase 1: LayerNorm + transpose + V projection (per t) --------
    with tc.tile_pool(name="xin", bufs=4) as xin_p, \
         tc.tile_pool(name="hnat", bufs=3) as h_p, \
         tc.tile_pool(name="stats", bufs=6) as st_p, \
         tc.tile_pool(name="ptr", bufs=3, space="PSUM") as ptr_p, \
         tc.tile_pool(name="psq", bufs=4, space="PSUM") as psq_p:
        for t in range(n_tt):
            xt = xin_p.tile([P, d], f32, tag="xt")
            nc.sync.dma_start(out=xt[:], in_=x_d[t * P:(t + 1) * P, :])
            st = st_p.tile([P, n_ch, 6], f32, tag="st")
            for c in range(n_ch):
                nc.vector.bn_stats(st[:, c, :], xt[:, c * bn_ch:(c + 1) * bn_ch])
            mv = st_p.tile([P, 2], f32, tag="mv")
            nc.vector.bn_aggr(mv[:], st[:])
            rstd = st_p.tile([P, 1], f32, tag="rstd")
            nc.scalar.activation(rstd[:], mv[:, 1:2], Act.Sqrt,
                                 bias=eps_sb[:], scale=1.0)
            nc.vector.reciprocal(rstd[:], rstd[:])
            ht = h_p.tile([P, d], f32, tag="ht")
            half = d // 2
            for c2 in range(2):
                nc.vector.tensor_scalar(out=ht[:, c2 * half:(c2 + 1) * half],
                                        in0=xt[:, c2 * half:(c2 + 1) * half],
                                        scalar1=mv[:, 0:1], scalar2=rstd[:],
                                        op0=Alu.subtract, op1=Alu.mult)
            for g in range(0, n_dt, 4):
                ng = min(4, n_dt - g)
                ps = ptr_p.tile([P, 512], f32, tag="ptr")
                for jj in range(ng):
                    nc.tensor.transpose(ps[:, jj * P:(jj + 1) * P],
                                        ht[:, (g + jj) * P:(g + jj + 1) * P],
                                        ident[:])
                nc.vector.tensor_copy(
                    out=hT[:, g:g + ng, t * P:(t + 1) * P],
                    in_=ps[:].rearrange("p (g q) -> p g q", q=P)[:, 0:ng, :])
            # V projection for this token tile (hT for all k just landed)
            vv = V[t][:].rearrange("p (h c) -> p h c", c=vw)
            nc.vector.memset(vv[:, :, hd:hd + 1], 1.0)
            psv = psq_p.tile([P, cc], f32, tag="psq")
            for k in range(n_dt):
                nc.tensor.matmul(psv[:], lhsT=hT[:, k, t * P:(t + 1) * P],
                                 rhs=wv_sb[k][:],
                                 start=(k == 0), stop=(k == n_dt - 1))
            nc.vector.tensor_add(vv[:, :, 0:hd],
                                 psv[:].rearrange("p (h c) -> p h c", c=hd),
                                 bv_bc[:].rearrange("p (h c) -> p h c", c=hd))

            # kT / qT for each completed 512-token block (overlaps next LN)
            if (t + 1) * P % FQ == 0:
                tb = ((t + 1) * P - FQ) // FQ
                for dst, w_sb, b_sb in ((kT, wk_sb, bk_sb), (qT, wq_sb, bq_sb)):
                    for j in range(n_ct):
                        ps = psq_p.tile([P, FQ], f32, tag="psq")
                        for k in range(n_dt):
                            nc.tensor.matmul(
                                ps[:], lhsT=w_sb[k][:, j * P:(j + 1) * P],
                                rhs=hT[:, k, tb * FQ:(tb + 1) * FQ],
                                start=(k == 0), stop=(k == n_dt - 1))
                        nc.vector.tensor_scalar_add(
                            out=dst[j][:, tb * FQ:(tb + 1) * FQ], in0=ps[:],
                            scalar1=b_sb[j][:, 0:1])

    # -------- phase 3+4: attention (head pairs, 1024-wide exp) + out-proj ----
    kt_pair = 2 if n_tt % 2 == 0 else 1
    with tc.tile_pool(name="pss", bufs=2, space="PSUM") as pss_p, \
         tc.tile_pool(name="pso", bufs=2, space="PSUM") as pso_p, \
         tc.tile_pool(name="psout", bufs=2, space="PSUM") as psout_p, \
         tc.tile_pool(name="expp", bufs=6) as exp_p, \
         tc.tile_pool(name="rsp", bufs=3) as rs_p, \
         tc.tile_pool(name="outp", bufs=3) as out_p:
        def outproj(qb):
            for tt in range(qb * FQ // P, (qb + 1) * FQ // P):
                for nb in range(n_nb):
                    ps = psout_p.tile([P, FN], f32, tag="pso2")
                    for j2 in range(n_ct):
                        nc.tensor.matmul(ps[:], lhsT=oT[j2][:, tt * P:(tt + 1) * P],
                                         rhs=wo_sb[j2][:, nb * FN:(nb + 1) * FN],
                                         start=(j2 == 0), stop=(j2 == n_ct - 1))
                    ot = out_p.tile([P, FN], f32, tag="ot")
                    nc.vector.tensor_copy(ot[:], ps[:])
                    nc.sync.dma_start(
                        out=out_d[tt * P:(tt + 1) * P, nb * FN:(nb + 1) * FN],
                        in_=ot[:])

        for qb in range(n_qb):
            for j in range(n_ct):
                if j == 1 and qb > 0:
                    outproj(qb - 1)   # overlaps this q-block's remaining pairs
                pos = [pso_p.tile([vw, FQ], f32, tag="po", name=f"po{hh}")
                       for hh in range(2)]
                for kt2 in range(n_tt // kt_pair):
                    exs = []
                    for hh in range(2):
                        off = hh * hd
                        ps = pss_p.tile([P, kt_pair * FQ], f32, tag="pss")
                        for u in range(kt_pair):
                            kt = kt_pair * kt2 + u
                            nc.tensor.matmul(
                                ps[:, u * FQ:(u + 1) * FQ],
                                lhsT=kT[j][off:off + hd, kt * P:(kt + 1) * P],
                                rhs=qT[j][off:off + hd, qb * FQ:(qb + 1) * FQ],
                                start=True, stop=True)
                        ex = exp_p.tile([P, kt_pair * FQ], bf16, tag="ex")
                        nc.scalar.activation(ex[:], ps[:], Act.Exp, scale=scale)
                        exs.append(ex)
                    for hh in range(2):
                        h = 2 * j + hh
                        for u in range(kt_pair):
                            kt = kt_pair * kt2 + u
                            nc.tensor.matmul(
                                pos[hh][:],
                                lhsT=V[kt][:, h * vw:(h + 1) * vw],
                                rhs=exs[hh][:, u * FQ:(u + 1) * FQ],
                                start=(kt == 0), stop=(kt == n_tt - 1))
                for hh in range(2):
                    h = 2 * j + hh
                    off = hh * hd
                    posb = rs_p.tile([vw, FQ], f32, tag="posb")
                    nc.vector.tensor_copy(posb[:], pos[hh][:])
                    rs = rs_p.tile([1, FQ], f32, tag="rs")
                    nc.vector.reciprocal(rs[:], posb[hd:hd + 1, :])
                    rsb = rs_p.tile([hd, FQ], f32, tag="rsb")
                    nc.sync.dma_start(out=rsb[:], in_=_bcast_partition(rs[:], hd))
                    nc.vector.tensor_mul(
                        oT[j][off:off + hd, qb * FQ:(qb + 1) * FQ],
                        posb[0:hd, :], rsb[:])
        outproj(n_qb - 1)


def build_nc(ntok=2048, d=1024, nh=8, hd=64, n_cores=8, repeat=1):
    nc = bacc.Bacc("TRN2", target_bir_lowering=False, debug=False,
                   num_devices=n_cores)
    f32 = mybir.dt.float32
    bf16 = mybir.dt.bfloat16
    cc = nh * hd
    x_d = nc.dram_tensor("x", [ntok, d], f32, kind="ExternalInput").ap()
    wq_d = nc.dram_tensor("wq", [d, cc], bf16, kind="ExternalInput").ap()
    wk_d = nc.dram_tensor("wk", [d, cc], bf16, kind="ExternalInput").ap()
    wv_d = nc.dram_tensor("wv", [d, cc], bf16, kind="ExternalInput").ap()
    wo_d = nc.dram_tensor("wo", [cc, d], bf16, kind="ExternalInput").ap()
    bq_d = nc.dram_tensor("bq", [cc], f32, kind="ExternalInput").ap()
    bk_d = nc.dram_tensor("bk", [cc], f32, kind="ExternalInput").ap()
    bv_d = nc.dram_tensor("bv", [cc], f32, kind="ExternalInput").ap()
    out_d = nc.dram_tensor("out", [ntok, d], f32, kind="ExternalOutput").ap()
    io = (x_d, wq_d, wk_d, wv_d, wo_d, bq_d, bk_d, bv_d, out_d)
    with tile.TileContext(nc) as tc:
        with ExitStack() as ctx:
            emit_body(ctx, tc, io, ntok, d, nh, hd, repeat=repeat)
    nc.compile()
    return nc


_CACHE = {}


def _make_runner(nc, n_cores):
    """Build a reusable sharded PJRT callable for the compiled Bass module.

    Mirrors concourse.bass2jax.run_bass_via_pjrt's multi-core path, but holds
    the jitted function so repeated kernel() calls (and timing loops) do not
    re-trace/re-compile the XLA wrapper.
    """
    import jax
    from jax.sharding import Mesh, PartitionSpec
    from jax.experimental.shard_map import shard_map
    from concourse.bass2jax import (_bass_exec_p, install_neuronx_cc_hook,
                                    partition_id_tensor)

    install_neuronx_cc_hook()
    partition_name = (nc.partition_id_tensor.name
                      if nc.partition_id_tensor else None)

    in_names, out_names, out_avals = [], [], []
    for alloc in nc.m.functions[0].allocations:
        if not isinstance(alloc, mybir.MemoryLocationSet):
            continue
        name = alloc.memorylocations[0].name
        if alloc.kind == "ExternalInput":
            if name != partition_name:
                in_names.append(name)
        elif alloc.kind == "ExternalOutput":
            out_names.append(name)
            out_avals.append(jax.core.ShapedArray(
                tuple(alloc.tensor_shape), mybir.dt.np(alloc.dtype)))
    all_names = in_names + out_names
    if partition_name is not None:
        all_names = all_names + [partition_name]

    def _body(*args):
        operands = list(args)
        if partition_name is not None:
            operands.append(partition_id_tensor())
        outs = _bass_exec_p.bind(
            *operands,
            out_avals=tuple(out_avals),
            in_names=tuple(all_names),
            out_names=tuple(out_names),
            lowering_input_output_aliases=(),
            sim_require_finite=True,
            sim_require_nnan=True,
            nc=nc,
        )
        return tuple(outs)

    devices = jax.devices()[:n_cores]
    assert len(devices) == n_cores
    mesh = Mesh(np.asarray(devices), ("core",))
    nio = len(in_names) + len(out_names)
    sharded = jax.jit(
        shard_map(_body, mesh=mesh,
                  in_specs=(PartitionSpec("core"),) * nio,
                  out_specs=(PartitionSpec("core"),) * len(out_names),
                  check_rep=False),
        keep_unused=True)
    return sharded, in_names, out_names, out_avals


def _concat_inputs(in_maps, in_names, out_avals, n_cores):
    concat_in = [np.concatenate([np.asarray(in_maps[c][name])
                                 for c in range(n_cores)], axis=0)
                 for name in in_names]
    concat_zeros = [np.zeros((n_cores * a.shape[0], *a.shape[1:]), a.dtype)
                    for a in out_avals]
    return concat_in + concat_zeros


def _run_spmd(in_maps, n_cores):
    sharded, in_names, out_names, out_avals = _CACHE["runner"]
    args = _concat_inputs(in_maps, in_names, out_avals, n_cores)
    _CACHE["last_args"] = args
    out_arrs = sharded(*args)
    return [
        {name: np.asarray(out_arrs[i]).reshape(n_cores, *out_avals[i].shape)[c]
         for i, name in enumerate(out_names)}
        for c in range(n_cores)
    ]


def kernel(x, ln_w, ln_b, w_qkv, w_out):
    x = np.asarray(x, dtype=np.float32)
    ln_w = np.asarray(ln_w, dtype=np.float32)
    ln_b = np.asarray(ln_b, dtype=np.float32)
    w_qkv = np.asarray(w_qkv, dtype=np.float32)
    w_out = np.asarray(w_out, dtype=np.float32)

    B, ntok, d = x.shape               # 4, 2048, 1024
    inner = w_out.shape[0]             # 1024
    hd = 64
    H = inner // hd                    # 16
    n_cores = 8
    gpb = n_cores // B                 # head-groups per batch (2)
    nh = H // gpb                      # heads per core (8)
    cc = nh * hd                       # 512

    if "nc" not in _CACHE:
        _CACHE["nc"] = build_nc(ntok=ntok, d=d, nh=nh, hd=hd, n_cores=n_cores)
    nc = _CACHE["nc"]

    bf = ml_dtypes.bfloat16
    # fold the LayerNorm affine into the projections (exact):
    #   h = (x - mu) * rstd * ln_w + ln_b
    #   h @ W = ((x - mu) * rstd) @ (diag(ln_w) W) + (ln_b @ W)
    wq_f = ln_w[:, None] * w_qkv[:, 0 * inner:1 * inner]
    wk_f = ln_w[:, None] * w_qkv[:, 1 * inner:2 * inner]
    wv_f = ln_w[:, None] * w_qkv[:, 2 * inner:3 * inner]
    bq_f = ln_b @ w_qkv[:, 0 * inner:1 * inner]
    bk_f = ln_b @ w_qkv[:, 1 * inner:2 * inner]
    bv_f = ln_b @ w_qkv[:, 2 * inner:3 * inner]

    in_maps = []
    for c in range(n_cores):
        b, g = divmod(c, gpb)
        cols = slice(g * cc, (g + 1) * cc)
        in_maps.append({
            "x": np.ascontiguousarray(x[b]),
            "wq": np.ascontiguousarray(wq_f[:, cols]).astype(bf),
            "wk": np.ascontiguousarray(wk_f[:, cols]).astype(bf),
            "wv": np.ascontiguousarray(wv_f[:, cols]).astype(bf),
            "wo": np.ascontiguousarray(w_out[cols, :]).astype(bf),
            "bq": np.ascontiguousarray(bq_f[cols]).astype(np.float32),
            "bk": np.ascontiguousarray(bk_f[cols]).astype(np.float32),
            "bv": np.ascontiguousarray(bv_f[cols]).astype(np.float32),
        })

    if "runner" not in _CACHE:
        _CACHE["runner"] = _make_runner(nc, n_cores)
    results = _run_spmd(in_maps, n_cores)
    parts = [results[c]["out"] for c in range(n_cores)]
    out = np.stack([sum(parts[b * gpb + g] for g in range(gpb))
                    for b in range(B)])
    return out.astype(np.float32)
